# revision 22
# baseline (speedup 1.0000x reference)
"""Bidirectional GATv2Conv (heads=1) on 8 Trainium2 NeuronCores.

Strategy (edge-parallel, dst-sharded -- no collectives):
- dst nodes range-sharded across 8 cores; each core owns every edge whose
  aggregation target is in its range, so segment-softmax stats stay local.
- Edges sorted by (block-pair, src-quarter, block); each (pair, quarter,
  block) run padded to tq tiles of 128 edges -> identical SPMD program.
- All PE/DVE data in bf16 (tolerance 2e-2 >> bf16 error):
    * node tables hold 128-wide rows [x | 1 | 0...]; plain dma_gather
      (elem 256B) yields G [128e, 128] whose col 64 is the ones column
      used to fold the softmax denominator into the scatter matmul.
    * gT via paired PE transposes packed 4-per-PSUM-bank, single
      [128,512] activation-copy escape (8 tiles per escape).
    * m computed c-major in packed PSUM banks (2 tiles per 128
      partitions); single Lrelu(alpha=0.2) escape per bank.
    * xr[dst] delivered via slot-major indicator: dstoff replicated
      across partitions by a stride-0 DMA (uint8), one DVE is_equal per
      512 cols; xr itself precomputed per-core into persistent SBUF.
    * logits batched per block into one PSUM tile; one exp per block.
    * scatter: indw = (iota==dstoff)*ex fused DVE op per tile; numerator
      and denominator accumulate via one [65,128] matmul per tile.
- out = (num/den) @ Wl + bias; Wl applied after aggregation
  (sum_e alpha_e * (x@Wl) == (sum_e alpha_e x) @ Wl).
"""

import numpy as np
from ml_dtypes import bfloat16

import concourse.bass as bass
import concourse.bacc as bacc
import concourse.mybir as mybir
import concourse.tile as tile
from concourse.bass import ds, AP
from concourse.bass_utils import run_bass_kernel_spmd

P = 128
NCORES = 8
NQ = 4           # src-table quarters (int16 idx limit: 32767 >= 25000)


def _ceil_div(a, b):
    return (a + b - 1) // b


def _prep_direction(x_dst, src, dst, ea, n_cores):
    """Per-core edge bucketing (before padding, which needs global TQ)."""
    N = x_dst.shape[0]
    npc = _ceil_div(N, n_cores)
    npc_pad = _ceil_div(npc, P) * P
    nblk = npc_pad // P
    cores = []
    for k in range(n_cores):
        lo = k * npc
        hi = min(lo + npc, N)
        sel = (dst >= lo) & (dst < hi)
        cores.append((src[sel], dst[sel] - lo, ea[sel]))
    return cores, npc, npc_pad, nblk


def _max_run(cores, qsize, nblk):
    m = 0
    for (e_src, e_dst, e_ea) in cores:
        blk = e_dst >> 7
        qua = e_src // qsize
        key = blk * NQ + qua
        cnt = np.bincount(key, minlength=nblk * NQ)
        m = max(m, int(cnt.max()))
    return m


def _layout_direction(cores, nblk, tq, qsize, de):
    """Build padded per-core device arrays (slot order: pair,quarter,block)."""
    npairs = nblk // 2
    run = tq * P
    slots_pair = NQ * 2 * run
    total = npairs * slots_pair
    nrun = npairs * NQ * 2
    out = []
    for (e_src, e_dst, e_ea) in cores:
        blk = e_dst >> 7
        qua = e_src // qsize
        runid = ((blk >> 1) * NQ + qua) * 2 + (blk & 1)
        order = np.argsort(runid, kind="stable")
        s_src = e_src[order]
        s_dst = e_dst[order]
        s_ea = e_ea[order]
        s_run = runid[order]
        s_loc = (s_src - (s_src // qsize) * qsize).astype(np.int16)
        s_off = (s_dst & 127).astype(np.uint8)

        idx_all = np.zeros(total, np.int16)
        doff_u8 = np.full(total, 255, np.uint8)
        doff_f = np.full(total, -1.0, np.float32)
        ea_all = np.zeros((total, de), np.float32)
        starts = np.searchsorted(s_run, np.arange(nrun + 1))
        for r in range(nrun):
            s0, s1 = int(starts[r]), int(starts[r + 1])
            cnt = s1 - s0
            assert cnt <= run, f"run {r} has {cnt} > {run} edges"
            base = r * run
            idx_all[base:base + cnt] = s_loc[s0:s1]
            doff_u8[base:base + cnt] = s_off[s0:s1]
            doff_f[base:base + cnt] = s_off[s0:s1]
            ea_all[base:base + cnt] = s_ea[s0:s1]

        idxw = np.tile(idx_all.reshape(-1, 16).T, (8, 1)).copy()
        doffC = doff_f.reshape(-1, P).T.copy()               # [128, total/128]
        eaT = np.ascontiguousarray(ea_all.T).astype(bfloat16)  # [de, total]
        doffR = doff_u8.reshape(1, total)
        out.append((idxw, doffC, eaT, doffR))
    return out


def _build_program(nblk, tq, npc_pad, qsize, de, d, c, unroll=False):
    assert nblk % 2 == 0
    npairs = nblk // 2
    ntile = NQ * tq
    slots_pair = NQ * 2 * tq * P
    fp = mybir.dt.float32
    bf = mybir.dt.bfloat16
    u8 = mybir.dt.uint8
    i16 = mybir.dt.int16
    AF = mybir.ActivationFunctionType
    ALU = mybir.AluOpType
    nc = bacc.Bacc("TRN2")

    def dram(name, shape, dt=fp, out=False):
        return nc.declare_dram_parameter(name, list(shape), dt, isOutput=out)

    dirs = {}
    for dn in ("b", "f"):
        dirs[dn] = dict(
            q_tabs=[dram(f"tab{q}_{dn}", [qsize, d]) for q in range(NQ)],
            idxw=dram(f"idxw_{dn}", [P, npairs * slots_pair // 16], i16),
            doffC=dram(f"doffC_{dn}", [P, npairs * 2 * ntile]),
            doffR=dram(f"doffR_{dn}", [1, npairs * slots_pair], u8),
            eaT=dram(f"eaT_{dn}", [de, npairs * slots_pair], bf),
            xdT=dram(f"xdT_{dn}", [d, npc_pad], bf),
            Wl=dram(f"Wl_{dn}", [d, c], bf),
            Wl2=dram(f"Wl2_{dn}", [2 * d, c], bf),
            Wr=dram(f"Wr_{dn}", [d, c], bf),
            We=dram(f"We_{dn}", [de, c], bf),
            att2=dram(f"att2_{dn}", [2 * c, 1], bf),
            bias=dram(f"bias_{dn}", [P, c]),
            out=dram(f"out_{dn}", [npc_pad, c], out=True),
        )
    iota_d = dram("iota", [P, P], bf)         # iota[p, j] = j
    iotac_d = dram("iotac", [P, 1])           # iotac[p] = p
    ident_d = dram("ident", [P, P], bf)
    ones_d = dram("ones", [P, 1], bf)

    # transpose slot list per block: (quarter, first tile, tiles in pair)
    tlist = []
    for q in range(NQ):
        for i0 in range(0, tq, 2):
            tlist.append((q, i0, min(2, tq - i0)))
    ntrans = len(tlist)
    tpq = _ceil_div(tq, 2)                      # transposes per quarter
    nbank_t = _ceil_div(ntrans, 4)              # gT psum banks per block
    nbank_m = _ceil_div(ntile, 8)               # m psum banks per block

    def t_slot(q, i):
        """tile (q, i) -> (gT bank, col, partition base)"""
        j = q * tpq + i // 2
        return j // 4, (j % 4) * P, (i % 2) * 64

    def m_slot(t):
        """tile t -> (m bank, partition base, col)"""
        return t // 8, (t % 2) * 64, ((t // 2) % 4) * P

    with tile.TileContext(nc) as tc:
        with tc.tile_pool(name="const", bufs=1) as cp, \
             tc.tile_pool(name="xr", bufs=1) as xp, \
             tc.tile_pool(name="load", bufs=2) as lp, \
             tc.tile_pool(name="gt", bufs=3) as gp, \
             tc.tile_pool(name="m8", bufs=3) as mp8, \
             tc.tile_pool(name="ind", bufs=20) as ip, \
             tc.tile_pool(name="work", bufs=4) as wp, \
             tc.tile_pool(name="indw", bufs=4) as iw, \
             tc.tile_pool(name="ps_t", bufs=3, space="PSUM") as ps_t, \
             tc.tile_pool(name="ps_m", bufs=3, space="PSUM") as ps_m, \
             tc.tile_pool(name="ps_s", bufs=2, space="PSUM") as ps_s:

            iota_t = cp.tile([P, P], bf)
            nc.sync.dma_start(out=iota_t[:], in_=iota_d[:])
            iotac_t = cp.tile([P, 1], fp)
            nc.sync.dma_start(out=iotac_t[:], in_=iotac_d[:])
            ident_t = cp.tile([P, P], bf)
            nc.sync.dma_start(out=ident_t[:], in_=ident_d[:])
            ones_t = cp.tile([P, 1], bf)
            nc.sync.dma_start(out=ones_t[:], in_=ones_d[:])

            for dn in ("b", "f"):
                dd = dirs[dn]
                Wl_t = cp.tile([d, c], bf, tag=f"Wl{dn}")
                nc.sync.dma_start(out=Wl_t[:], in_=dd["Wl"][:])
                Wl2_t = cp.tile([2 * d, c], bf, tag=f"Wl2{dn}")
                nc.sync.dma_start(out=Wl2_t[:], in_=dd["Wl2"][:])
                Wr_t = cp.tile([d, c], bf, tag=f"Wr{dn}")
                nc.sync.dma_start(out=Wr_t[:], in_=dd["Wr"][:])
                We_t = cp.tile([de, c], bf, tag=f"We{dn}")
                nc.sync.dma_start(out=We_t[:], in_=dd["We"][:])
                att2_t = cp.tile([2 * c, 1], bf, tag=f"att2{dn}")
                nc.sync.dma_start(out=att2_t[:], in_=dd["att2"][:])
                bias_t = cp.tile([P, c], fp, tag=f"bias{dn}")
                nc.sync.dma_start(out=bias_t[:], in_=dd["bias"][:])

                # ---- prologue: xr = x_dst @ Wr for all own blocks ----
                xr_sb = xp.tile([P, nblk * c], bf, tag=f"xr{dn}")
                for g in range(_ceil_div(nblk, 8)):
                    nb = min(8, nblk - 8 * g)
                    xd_t = lp.tile([d, 8 * P], bf, tag="xd")
                    nc.sync.dma_start(out=xd_t[:, 0:nb * P],
                                      in_=dd["xdT"][:, ds(g * 8 * P, nb * P)])
                    pro_ps = ps_m.tile([P, 512], fp, tag="mb")
                    for j in range(nb):
                        nc.tensor.matmul(out=pro_ps[:, j * c:(j + 1) * c],
                                         lhsT=xd_t[:, j * P:(j + 1) * P],
                                         rhs=Wr_t[:], start=True, stop=True)
                    nc.scalar.activation(out=xr_sb[:, ds(g * 8 * c, nb * c)],
                                         in_=pro_ps[:, 0:nb * c], func=AF.Copy)

                def pair_body(pv):
                    base = pv * slots_pair
                    idxw_t = lp.tile([P, slots_pair // 16], i16, tag="idxw")
                    nc.sync.dma_start(
                        out=idxw_t[:],
                        in_=dd["idxw"][:, ds(pv * (slots_pair // 16),
                                             slots_pair // 16)])
                    ea_t = lp.tile([de, slots_pair], bf, tag="ea")
                    nc.sync.dma_start(out=ea_t[:],
                                      in_=dd["eaT"][:, ds(base, slots_pair)])
                    dC_t = lp.tile([P, 2 * ntile], fp, tag="dC")
                    nc.sync.dma_start(out=dC_t[:],
                                      in_=dd["doffC"][:, ds(pv * 2 * ntile,
                                                            2 * ntile)])
                    dR_t = lp.tile([P, slots_pair], u8, tag="dR")
                    src = dd["doffR"][0:1, ds(base, slots_pair)]
                    rep = AP(src.tensor, src.offset, [[0, P]] + src.ap[1:])
                    nc.sync.dma_start(out=dR_t[:], in_=rep)

                    xr_pair = lp.tile([P, 2 * c], bf, tag="xrp")
                    nc.sync.dma_start(out=xr_pair[:],
                                      in_=xr_sb[:, ds(pv * 2 * c, 2 * c)])
                    G_ts = []
                    for q in range(NQ):
                        G32 = lp.tile([P, 2 * tq, d], fp, tag=f"G32_{q}")
                        nc.gpsimd.dma_gather(
                            out_ap=G32[:],
                            in_ap=dd["q_tabs"][q][:],
                            idxs_ap=idxw_t[:, q * (2 * tq * 8):
                                           (q + 1) * (2 * tq * 8)],
                            num_idxs=2 * tq * P,
                            num_idxs_reg=2 * tq * P,
                            elem_size=d,
                        )
                        G = lp.tile([P, 2 * tq, c + 1], bf, tag=f"G{q}")
                        nc.vector.memset(G[:, :, c:c + 1], 1.0)
                        nc.vector.tensor_scalar_mul(out=G[:, :, 0:c],
                                                    in0=G32[:, :, :],
                                                    scalar1=1.0)
                        G_ts.append(G)

                    # slot-major indicator, one tile per (quarter, block)
                    # run, block-0 runs emitted first so m(0) is fed early
                    run_sz = tq * P
                    indT_ch = [None] * (2 * NQ)
                    for run in [q * 2 + b for b in (0, 1) for q in range(NQ)]:
                        it = ip.tile([P, run_sz], bf, tag="indT", name="indT")
                        nc.vector.tensor_scalar(
                            out=it[:],
                            in0=dR_t[:, ds(run * run_sz, run_sz)],
                            scalar1=iotac_t[:, 0:1], scalar2=None,
                            op0=ALU.is_equal)
                        indT_ch[run] = it

                    # skewed per-block phases: block 1's transposes are
                    # emitted between block 0's attention and scatter so PE
                    # stays busy while exp/indw cook on ACT/DVE.
                    gT8s = {}
                    m8s = {}
                    ex_ts = {}
                    S_ts = {}
                    indw_ts = {}

                    def do_trans(b):
                        for k in range(nbank_t):
                            tbank = ps_t.tile([P, 512], bf, tag="tb", name="tb")
                            for j, (q, i0, w) in enumerate(tlist):
                                if j // 4 != k:
                                    continue
                                for u in range(w):
                                    nc.tensor.transpose(
                                        out=tbank[u * 64:u * 64 + 64,
                                                  ds((j % 4) * P, P)],
                                        in_=G_ts[q][:, b * tq + i0 + u, 0:64],
                                        identity=ident_t[:])
                            g8 = gp.tile([P, 512], bf, tag="g8")
                            nc.scalar.activation(out=g8[:], in_=tbank[:],
                                                 func=AF.Copy)
                            gT8s[(b, k)] = g8

                    def do_m(b):
                        mbanks = [ps_m.tile([P, 512], fp, tag="mb", name="mb")
                                  for _ in range(nbank_m)]
                        for t in range(ntile):
                            q, i = t // tq, t % tq
                            tb, tcol, tpb = t_slot(q, i)
                            mb, mpb, mcol = m_slot(t)
                            dst = mbanks[mb][mpb:mpb + c, ds(mcol, P)]
                            rhs_g = gT8s[(b, tb)][tpb:tpb + 64, ds(tcol, P)]
                            scol = ((q * 2 + b) * tq + i) * P
                            nc.tensor.matmul(out=dst,
                                             lhsT=Wl2_t[tpb:tpb + d, :],
                                             rhs=rhs_g,
                                             start=True, stop=False)
                            nc.tensor.matmul(out=dst, lhsT=We_t[:],
                                             rhs=ea_t[:, ds(scol, P)],
                                             start=False, stop=False)
                            nc.tensor.matmul(out=dst,
                                             lhsT=xr_pair[:, b * c:(b + 1) * c],
                                             rhs=indT_ch[q * 2 + b]
                                             [:, ds(i * P, P)],
                                             start=False, stop=True)
                            if (t + 1) % 8 == 0 or t == ntile - 1:
                                k = t // 8
                                n_t = min(8, ntile - 8 * k)
                                cols = _ceil_div(n_t, 2) * P
                                m8 = mp8.tile([P, 512], bf, tag="m8")
                                nc.scalar.activation(out=m8[:, 0:cols],
                                                     in_=mbanks[k][:, 0:cols],
                                                     func=AF.Prelu, alpha=0.2)
                                m8s[(b, k)] = m8

                    def do_attn(b):
                        S = ps_s.tile([P, 512], fp, tag="sb", name="sb")
                        S_ts[b] = S
                        lg_ps = S[:, 0:ntile]
                        for t in range(ntile):
                            mb, mpb, mcol = m_slot(t)
                            nc.tensor.matmul(
                                out=lg_ps[:, t:t + 1],
                                lhsT=m8s[(b, mb)][mpb:mpb + c, ds(mcol, P)],
                                rhs=att2_t[mpb:mpb + c, :],
                                start=True, stop=True)
                        ex_t = wp.tile([P, ntile], fp, tag="ex")
                        nc.scalar.activation(out=ex_t[:], in_=lg_ps[:],
                                             func=AF.Exp)
                        ex_ts[b] = ex_t
                        for t in range(ntile):
                            q, i = t // tq, t % tq
                            indw = iw.tile([P, P], bf, tag="iw")
                            nc.vector.tensor_scalar(
                                out=indw[:], in0=iota_t[:],
                                scalar1=dC_t[:, (q * 2 + b) * tq + i:
                                             (q * 2 + b) * tq + i + 1],
                                scalar2=ex_t[:, t:t + 1],
                                op0=ALU.is_equal, op1=ALU.mult)
                            indw_ts[(b, t)] = indw

                    def do_scatter(b):
                        S = S_ts[b]
                        blk_ps = S[0:65, 256:384]
                        for t in range(ntile):
                            q, i = t // tq, t % tq
                            nc.tensor.matmul(
                                out=blk_ps,
                                lhsT=G_ts[q][:, b * tq + i, 0:c + 1],
                                rhs=indw_ts[(b, t)][:],
                                start=(t == 0), stop=(t == ntile - 1))
                        bsb = wp.tile([65, P], bf, tag="bsb")
                        nc.scalar.activation(out=bsb[:], in_=blk_ps,
                                             func=AF.Copy)
                        denc_ps = S[:, 384:385]
                        nc.tensor.matmul(out=denc_ps, lhsT=bsb[64:65, :],
                                         rhs=ones_t[64:65, :],
                                         start=True, stop=True)
                        post_ps = S[:, 448:448 + c]
                        nc.tensor.matmul(out=post_ps, lhsT=bsb[0:64, :],
                                         rhs=Wl_t[:], start=True, stop=True)
                        dpe = wp.tile([P, 1], fp, tag="dpe")
                        nc.vector.tensor_scalar_add(out=dpe[:], in0=denc_ps,
                                                    scalar1=1e-16)
                        rec = wp.tile([P, 1], fp, tag="rec")
                        nc.vector.reciprocal(out=rec[:], in_=dpe[:])
                        sc = wp.tile([P, c], fp, tag="sc")
                        nc.vector.tensor_scalar(out=sc[:], in0=post_ps,
                                                scalar1=rec[:, 0:1],
                                                scalar2=None, op0=ALU.mult)
                        outt = wp.tile([P, c], fp, tag="outt")
                        nc.vector.tensor_tensor(out=outt[:], in0=sc[:],
                                                in1=bias_t[:], op=ALU.add)
                        nc.sync.dma_start(
                            out=dd["out"][ds((2 * pv + b) * P, P), :],
                            in_=outt[:])

                    do_trans(0)
                    do_m(0)
                    do_attn(0)
                    do_trans(1)
                    do_scatter(0)
                    do_m(1)
                    do_attn(1)
                    do_scatter(1)

                if unroll:
                    for pv in range(npairs):
                        pair_body(pv)
                else:
                    CHUNK = 12
                    for s0 in range(0, npairs, CHUNK):
                        with tc.For_i(s0, min(s0 + CHUNK, npairs), 1,
                                      staggered_reset=True) as pv:
                            pair_body(pv)

    nc.compile()
    return nc, dirs


def kernel(x0, x1, edge_index, edge_attr,
           Wl_b, Wr_b, We_b, att_b, b_b,
           Wl_f, Wr_f, We_f, att_f, b_f):
    x0 = np.asarray(x0, np.float32)
    x1 = np.asarray(x1, np.float32)
    edge_attr = np.asarray(edge_attr, np.float32)
    ei = np.asarray(edge_index)
    src, dst = ei[0].astype(np.int64), ei[1].astype(np.int64)

    N, d = x0.shape
    de = edge_attr.shape[1]
    c = np.asarray(Wl_b).shape[1]
    qsize = _ceil_div(N, NQ)
    assert qsize <= 32767

    cores_b, npc, npc_pad, nblk = _prep_direction(x1, src, dst, edge_attr, NCORES)
    cores_f, _, _, _ = _prep_direction(x0, dst, src, edge_attr, NCORES)

    tq = _ceil_div(max(_max_run(cores_b, qsize, nblk),
                       _max_run(cores_f, qsize, nblk)), P)

    lay_b = _layout_direction(cores_b, nblk, tq, qsize, de)
    lay_f = _layout_direction(cores_f, nblk, tq, qsize, de)

    nc, dirs = _build_program(nblk, tq, npc_pad, qsize, de, d, c)

    def tabs(x):
        t = []
        for q in range(NQ):
            xx = x[q * qsize:(q + 1) * qsize]
            rows = np.zeros((qsize, d), np.float32)
            rows[:xx.shape[0]] = xx
            t.append(rows)
        return t

    x0t, x1t = tabs(x0), tabs(x1)
    iota = np.broadcast_to(np.arange(P, dtype=np.float32)[None, :],
                           (P, P)).astype(bfloat16).copy()
    iotac = np.arange(P, dtype=np.float32).reshape(P, 1).copy()
    ident = np.eye(P, dtype=np.float32).astype(bfloat16)
    ones = np.ones((P, 1), np.float32).astype(bfloat16)

    def xdT_shard(xd, k):
        lo = k * npc
        sh = xd[lo:min(lo + npc, N)]
        pad = np.zeros((npc_pad, d), np.float32)
        pad[:sh.shape[0]] = sh
        return np.ascontiguousarray(pad.T).astype(bfloat16)

    in_maps = []
    for k in range(NCORES):
        (idxw_b, doffC_b, eaT_b, doffR_b) = lay_b[k]
        (idxw_f, doffC_f, eaT_f, doffR_f) = lay_f[k]
        m = {
            "iota": iota, "iotac": iotac, "ident": ident, "ones": ones,
            "idxw_b": idxw_b, "doffC_b": doffC_b, "eaT_b": eaT_b,
            "doffR_b": doffR_b,
            "idxw_f": idxw_f, "doffC_f": doffC_f, "eaT_f": eaT_f,
            "doffR_f": doffR_f,
            "xdT_b": xdT_shard(x1, k), "xdT_f": xdT_shard(x0, k),
            "Wl_b": np.asarray(Wl_b, np.float32).astype(bfloat16),
            "Wl2_b": np.tile(np.asarray(Wl_b, np.float32), (2, 1)).astype(bfloat16),
            "Wr_b": np.asarray(Wr_b, np.float32).astype(bfloat16),
            "We_b": np.asarray(We_b, np.float32).astype(bfloat16),
            "att2_b": np.tile(np.asarray(att_b, np.float32).reshape(c, 1),
                              (2, 1)).astype(bfloat16),
            "bias_b": np.broadcast_to(np.asarray(b_b, np.float32)[None, :],
                                      (P, c)).copy(),
            "Wl_f": np.asarray(Wl_f, np.float32).astype(bfloat16),
            "Wl2_f": np.tile(np.asarray(Wl_f, np.float32), (2, 1)).astype(bfloat16),
            "Wr_f": np.asarray(Wr_f, np.float32).astype(bfloat16),
            "We_f": np.asarray(We_f, np.float32).astype(bfloat16),
            "att2_f": np.tile(np.asarray(att_f, np.float32).reshape(c, 1),
                              (2, 1)).astype(bfloat16),
            "bias_f": np.broadcast_to(np.asarray(b_f, np.float32)[None, :],
                                      (P, c)).copy(),
        }
        for q in range(NQ):
            m[f"tab{q}_b"] = x0t[q]
            m[f"tab{q}_f"] = x1t[q]
        in_maps.append(m)

    kernel.last_tq, kernel.last_nblk = tq, nblk
    kernel.last_npc_pad, kernel.last_qsize = npc_pad, qsize
    res = run_bass_kernel_spmd(nc, in_maps, list(range(NCORES)))

    out_b = np.concatenate([res.results[k]["out_b"][:npc]
                            for k in range(NCORES)])[:N]
    out_f = np.concatenate([res.results[k]["out_f"][:npc]
                            for k in range(NCORES)])[:N]
    return (out_b, out_f)


# revision 23
# speedup vs baseline: 5.5944x; 5.5944x over previous
"""Bidirectional GATv2Conv (heads=1) on 8 Trainium2 NeuronCores.

Strategy (edge-parallel, dst-sharded -- no collectives):
- dst nodes range-sharded across 8 cores; each core owns every edge whose
  aggregation target is in its range, so segment-softmax stats stay local.
- Edges sorted by (block-pair, src-quarter, block); each (pair, quarter,
  block) run padded to tq tiles of 128 edges -> identical SPMD program.
- All PE/DVE data in bf16 (tolerance 2e-2 >> bf16 error):
    * node tables hold 128-wide rows [x | 1 | 0...]; plain dma_gather
      (elem 256B) yields G [128e, 128] whose col 64 is the ones column
      used to fold the softmax denominator into the scatter matmul.
    * gT via paired PE transposes packed 4-per-PSUM-bank, single
      [128,512] activation-copy escape (8 tiles per escape).
    * m computed c-major in packed PSUM banks (2 tiles per 128
      partitions); single Lrelu(alpha=0.2) escape per bank.
    * xr[dst] delivered via slot-major indicator: dstoff replicated
      across partitions by a stride-0 DMA (uint8), one DVE is_equal per
      512 cols; xr itself precomputed per-core into persistent SBUF.
    * logits batched per block into one PSUM tile; one exp per block.
    * scatter: indw = (iota==dstoff)*ex fused DVE op per tile; numerator
      and denominator accumulate via one [65,128] matmul per tile.
- out = (num/den) @ Wl + bias; Wl applied after aggregation
  (sum_e alpha_e * (x@Wl) == (sum_e alpha_e x) @ Wl).
"""

import numpy as np
from ml_dtypes import bfloat16

import concourse.bass as bass
import concourse.bacc as bacc
import concourse.mybir as mybir
import concourse.tile as tile
from concourse.bass import ds, AP
from concourse.bass_utils import run_bass_kernel_spmd

P = 128
NCORES = 8
NQ = 4           # src-table quarters (int16 idx limit: 32767 >= 25000)


def _ceil_div(a, b):
    return (a + b - 1) // b


def _prep_direction(x_dst, src, dst, ea, n_cores):
    """Per-core edge bucketing (before padding, which needs global TQ)."""
    N = x_dst.shape[0]
    npc = _ceil_div(N, n_cores)
    npc_pad = _ceil_div(npc, P) * P
    nblk = npc_pad // P
    cores = []
    for k in range(n_cores):
        lo = k * npc
        hi = min(lo + npc, N)
        sel = (dst >= lo) & (dst < hi)
        cores.append((src[sel], dst[sel] - lo, ea[sel]))
    return cores, npc, npc_pad, nblk


def _max_run(cores, qsize, nblk):
    m = 0
    for (e_src, e_dst, e_ea) in cores:
        blk = e_dst >> 7
        qua = e_src // qsize
        key = blk * NQ + qua
        cnt = np.bincount(key, minlength=nblk * NQ)
        m = max(m, int(cnt.max()))
    return m


def _layout_direction(cores, nblk, tq, qsize, de):
    """Build padded per-core device arrays (slot order: pair,quarter,block)."""
    npairs = nblk // 2
    run = tq * P
    slots_pair = NQ * 2 * run
    total = npairs * slots_pair
    nrun = npairs * NQ * 2
    out = []
    for (e_src, e_dst, e_ea) in cores:
        blk = e_dst >> 7
        qua = e_src // qsize
        runid = ((blk >> 1) * NQ + qua) * 2 + (blk & 1)
        order = np.argsort(runid, kind="stable")
        s_src = e_src[order]
        s_dst = e_dst[order]
        s_ea = e_ea[order]
        s_run = runid[order]
        s_loc = (s_src - (s_src // qsize) * qsize).astype(np.int16)
        s_off = (s_dst & 127).astype(np.uint8)

        idx_all = np.zeros(total, np.int16)
        doff_u8 = np.full(total, 255, np.uint8)
        doff_f = np.full(total, -1.0, np.float32)
        ea_all = np.zeros((total, de), np.float32)
        starts = np.searchsorted(s_run, np.arange(nrun + 1))
        for r in range(nrun):
            s0, s1 = int(starts[r]), int(starts[r + 1])
            cnt = s1 - s0
            assert cnt <= run, f"run {r} has {cnt} > {run} edges"
            base = r * run
            idx_all[base:base + cnt] = s_loc[s0:s1]
            doff_u8[base:base + cnt] = s_off[s0:s1]
            doff_f[base:base + cnt] = s_off[s0:s1]
            ea_all[base:base + cnt] = s_ea[s0:s1]

        idxw = np.tile(idx_all.reshape(-1, 16).T, (8, 1)).copy()
        doffC = doff_f.reshape(-1, P).T.copy()               # [128, total/128]
        eaT = np.ascontiguousarray(ea_all.T).astype(bfloat16)  # [de, total]
        doffR = doff_u8.reshape(1, total)
        out.append((idxw, doffC, eaT, doffR))
    return out


def _build_program(nblk, tq, npc_pad, qsize, de, d, c, unroll=False):
    assert nblk % 2 == 0
    npairs = nblk // 2
    ntile = NQ * tq
    slots_pair = NQ * 2 * tq * P
    fp = mybir.dt.float32
    bf = mybir.dt.bfloat16
    u8 = mybir.dt.uint8
    i16 = mybir.dt.int16
    AF = mybir.ActivationFunctionType
    ALU = mybir.AluOpType
    nc = bacc.Bacc("TRN2")

    def dram(name, shape, dt=fp, out=False):
        return nc.declare_dram_parameter(name, list(shape), dt, isOutput=out)

    dirs = {}
    for dn in ("b", "f"):
        dirs[dn] = dict(
            q_tabs=[dram(f"tab{q}_{dn}", [qsize, d]) for q in range(NQ)],
            idxw=dram(f"idxw_{dn}", [P, npairs * slots_pair // 16], i16),
            doffC=dram(f"doffC_{dn}", [P, npairs * 2 * ntile]),
            doffR=dram(f"doffR_{dn}", [1, npairs * slots_pair], u8),
            eaT=dram(f"eaT_{dn}", [de, npairs * slots_pair], bf),
            xdT=dram(f"xdT_{dn}", [d, npc_pad], bf),
            out=dram(f"out_{dn}", [npc_pad, c], out=True),
        )
    cstb_d = dram("cstb", [P, 771], bf)   # packed bf16 consts
    cstf_d = dram("cstf", [P, 129])       # packed fp32 consts

    # transpose slot list per block: (quarter, first tile, tiles in pair)
    tlist = []
    for q in range(NQ):
        for i0 in range(0, tq, 2):
            tlist.append((q, i0, min(2, tq - i0)))
    ntrans = len(tlist)
    tpq = _ceil_div(tq, 2)                      # transposes per quarter
    nbank_t = _ceil_div(ntrans, 4)              # gT psum banks per block
    nbank_m = _ceil_div(ntile, 8)               # m psum banks per block

    def t_slot(q, i):
        """tile (q, i) -> (gT bank, col, partition base)"""
        j = q * tpq + i // 2
        return j // 4, (j % 4) * P, (i % 2) * 64

    def m_slot(t):
        """tile t -> (m bank, partition base, col)"""
        return t // 8, (t % 2) * 64, ((t // 2) % 4) * P

    with tile.TileContext(nc) as tc:
        with tc.tile_pool(name="const", bufs=1) as cp, \
             tc.tile_pool(name="xr", bufs=1) as xp, \
             tc.tile_pool(name="load", bufs=2) as lp, \
             tc.tile_pool(name="gt", bufs=3) as gp, \
             tc.tile_pool(name="m8", bufs=3) as mp8, \
             tc.tile_pool(name="ind", bufs=20) as ip, \
             tc.tile_pool(name="work", bufs=4) as wp, \
             tc.tile_pool(name="indw", bufs=4) as iw, \
             tc.tile_pool(name="ps_t", bufs=3, space="PSUM") as ps_t, \
             tc.tile_pool(name="ps_m", bufs=3, space="PSUM") as ps_m, \
             tc.tile_pool(name="ps_s", bufs=2, space="PSUM") as ps_s:

            cstb_t = cp.tile([P, 771], bf)
            nc.scalar.dma_start(out=cstb_t[:], in_=cstb_d[:])
            cstf_t = cp.tile([P, 129], fp)
            nc.scalar.dma_start(out=cstf_t[:], in_=cstf_d[:])
            iota_t = cstb_t[:, 0:P]
            ident_t = cstb_t[:, P:2 * P]
            ones_t = cstb_t[:, 2 * P:2 * P + 1]
            iotac_t = cstf_t[:, 0:1]

            bodies = []
            for di, dn in enumerate(("b", "f")):
                dd = dirs[dn]
                o = 257 + di * 257
                Wl_t = cstb_t[0:d, o:o + c]
                Wl2_t = cstb_t[:, o + c:o + 2 * c]
                Wr_t = cstb_t[0:d, o + 2 * c:o + 3 * c]
                We_t = cstb_t[0:de, o + 3 * c:o + 4 * c]
                att2_t = cstb_t[:, o + 4 * c:o + 4 * c + 1]
                bias_t = cstf_t[:, 1 + di * c:1 + (di + 1) * c]

                # ---- prologue: xr = x_dst @ Wr for all own blocks ----
                xr_sb = xp.tile([P, nblk * c], bf, tag=f"xr{dn}")
                for g in range(_ceil_div(nblk, 8)):
                    nb = min(8, nblk - 8 * g)
                    xd_t = lp.tile([d, 8 * P], bf, tag="xd")
                    nc.scalar.dma_start(out=xd_t[:, 0:nb * P],
                                        in_=dd["xdT"][:, ds(g * 8 * P, nb * P)])
                    pro_ps = ps_m.tile([P, 512], fp, tag="mb")
                    for j in range(nb):
                        nc.tensor.matmul(out=pro_ps[:, j * c:(j + 1) * c],
                                         lhsT=xd_t[:, j * P:(j + 1) * P],
                                         rhs=Wr_t, start=True, stop=True)
                    nc.scalar.activation(out=xr_sb[:, ds(g * 8 * c, nb * c)],
                                         in_=pro_ps[:, 0:nb * c], func=AF.Copy)

                def pair_body(pv, dd=dd, Wl_t=Wl_t, Wl2_t=Wl2_t,
                              We_t=We_t, att2_t=att2_t, bias_t=bias_t,
                              xr_sb=xr_sb):
                    base = pv * slots_pair
                    idxw_t = lp.tile([P, slots_pair // 16], i16, tag="idxw")
                    nc.sync.dma_start(
                        out=idxw_t[:],
                        in_=dd["idxw"][:, ds(pv * (slots_pair // 16),
                                             slots_pair // 16)])
                    ea_t = lp.tile([de, slots_pair], bf, tag="ea")
                    nc.sync.dma_start(out=ea_t[:],
                                      in_=dd["eaT"][:, ds(base, slots_pair)])
                    dC_t = lp.tile([P, 2 * ntile], fp, tag="dC")
                    nc.sync.dma_start(out=dC_t[:],
                                      in_=dd["doffC"][:, ds(pv * 2 * ntile,
                                                            2 * ntile)])
                    dR_t = lp.tile([P, slots_pair], u8, tag="dR")
                    src = dd["doffR"][0:1, ds(base, slots_pair)]
                    rep = AP(src.tensor, src.offset, [[0, P]] + src.ap[1:])
                    nc.sync.dma_start(out=dR_t[:], in_=rep)

                    xr_pair = lp.tile([P, 2 * c], bf, tag="xrp")
                    nc.sync.dma_start(out=xr_pair[:],
                                      in_=xr_sb[:, ds(pv * 2 * c, 2 * c)])
                    G_ts = []
                    for q in range(NQ):
                        G32 = lp.tile([P, 2 * tq, d], fp, tag=f"G32_{q}")
                        for b in (0, 1):
                            run = q * 2 + b
                            nc.gpsimd.dma_gather(
                                out_ap=G32[:, b * tq:(b + 1) * tq, :],
                                in_ap=dd["q_tabs"][q][:],
                                idxs_ap=idxw_t[:, run * (tq * 8):
                                               (run + 1) * (tq * 8)],
                                num_idxs=tq * P,
                                num_idxs_reg=tq * P,
                                elem_size=d,
                            )
                        G = lp.tile([P, 2 * tq, c + 1], bf, tag=f"G{q}")
                        nc.vector.memset(G[:, :, c:c + 1], 1.0)
                        nc.vector.tensor_scalar_mul(out=G[:, :, 0:c],
                                                    in0=G32[:, :, :],
                                                    scalar1=1.0)
                        G_ts.append(G)

                    # slot-major indicator, one tile per (quarter, block)
                    # run, block-0 runs emitted first so m(0) is fed early
                    run_sz = tq * P
                    indT_ch = [None] * (2 * NQ)
                    for run in [q * 2 + b for b in (0, 1) for q in range(NQ)]:
                        it = ip.tile([P, run_sz], bf, tag="indT", name="indT")
                        nc.vector.tensor_scalar(
                            out=it[:],
                            in0=dR_t[:, ds(run * run_sz, run_sz)],
                            scalar1=iotac_t, scalar2=None,
                            op0=ALU.is_equal)
                        indT_ch[run] = it

                    # skewed per-block phases: block 1's transposes are
                    # emitted between block 0's attention and scatter so PE
                    # stays busy while exp/indw cook on ACT/DVE.
                    gT8s = {}
                    m8s = {}
                    ex_ts = {}
                    S_ts = {}
                    indw_ts = {}

                    def do_trans(b):
                        for k in range(nbank_t):
                            tbank = ps_t.tile([P, 512], bf, tag="tb", name="tb")
                            for j, (q, i0, w) in enumerate(tlist):
                                if j // 4 != k:
                                    continue
                                for u in range(w):
                                    nc.tensor.transpose(
                                        out=tbank[u * 64:u * 64 + 64,
                                                  ds((j % 4) * P, P)],
                                        in_=G_ts[q][:, b * tq + i0 + u, 0:64],
                                        identity=ident_t)
                            g8 = gp.tile([P, 512], bf, tag="g8")
                            nc.scalar.activation(out=g8[:], in_=tbank[:],
                                                 func=AF.Copy)
                            gT8s[(b, k)] = g8

                    def do_m(b):
                        mbanks = [ps_m.tile([P, 512], fp, tag="mb", name="mb")
                                  for _ in range(nbank_m)]
                        for t in range(ntile):
                            q, i = t // tq, t % tq
                            tb, tcol, tpb = t_slot(q, i)
                            mb, mpb, mcol = m_slot(t)
                            dst = mbanks[mb][mpb:mpb + c, ds(mcol, P)]
                            rhs_g = gT8s[(b, tb)][tpb:tpb + 64, ds(tcol, P)]
                            scol = ((q * 2 + b) * tq + i) * P
                            nc.tensor.matmul(out=dst,
                                             lhsT=Wl2_t[tpb:tpb + d, :],
                                             rhs=rhs_g,
                                             start=True, stop=False)
                            nc.tensor.matmul(out=dst, lhsT=We_t,
                                             rhs=ea_t[:, ds(scol, P)],
                                             start=False, stop=False)
                            nc.tensor.matmul(out=dst,
                                             lhsT=xr_pair[:, b * c:(b + 1) * c],
                                             rhs=indT_ch[q * 2 + b]
                                             [:, ds(i * P, P)],
                                             start=False, stop=True)
                            if (t + 1) % 8 == 0 or t == ntile - 1:
                                k = t // 8
                                n_t = min(8, ntile - 8 * k)
                                cols = _ceil_div(n_t, 2) * P
                                m8 = mp8.tile([P, 512], bf, tag="m8")
                                nc.scalar.activation(out=m8[:, 0:cols],
                                                     in_=mbanks[k][:, 0:cols],
                                                     func=AF.Prelu, alpha=0.2)
                                m8s[(b, k)] = m8

                    def do_attn(b):
                        S = ps_s.tile([P, 512], fp, tag="sb", name="sb")
                        S_ts[b] = S
                        lg_ps = S[:, 0:ntile]
                        for t in range(ntile):
                            mb, mpb, mcol = m_slot(t)
                            nc.tensor.matmul(
                                out=lg_ps[:, t:t + 1],
                                lhsT=m8s[(b, mb)][mpb:mpb + c, ds(mcol, P)],
                                rhs=att2_t[mpb:mpb + c, :],
                                start=True, stop=True)
                        ex_t = wp.tile([P, ntile], fp, tag="ex")
                        nc.scalar.activation(out=ex_t[:], in_=lg_ps[:],
                                             func=AF.Exp)
                        ex_ts[b] = ex_t
                        for t in range(ntile):
                            q, i = t // tq, t % tq
                            indw = iw.tile([P, P], bf, tag="iw")
                            nc.vector.tensor_scalar(
                                out=indw[:], in0=iota_t,
                                scalar1=dC_t[:, (q * 2 + b) * tq + i:
                                             (q * 2 + b) * tq + i + 1],
                                scalar2=ex_t[:, t:t + 1],
                                op0=ALU.is_equal, op1=ALU.mult)
                            indw_ts[(b, t)] = indw

                    def do_scatter(b):
                        S = S_ts[b]
                        blk_ps = S[0:65, 256:384]
                        for t in range(ntile):
                            q, i = t // tq, t % tq
                            nc.tensor.matmul(
                                out=blk_ps,
                                lhsT=G_ts[q][:, b * tq + i, 0:c + 1],
                                rhs=indw_ts[(b, t)][:],
                                start=(t == 0), stop=(t == ntile - 1))
                        bsb = wp.tile([65, P], bf, tag="bsb")
                        nc.scalar.activation(out=bsb[:], in_=blk_ps,
                                             func=AF.Copy)
                        denc_ps = S[:, 384:385]
                        nc.tensor.matmul(out=denc_ps, lhsT=bsb[64:65, :],
                                         rhs=ones_t[64:65, 0:1],
                                         start=True, stop=True)
                        post_ps = S[:, 448:448 + c]
                        nc.tensor.matmul(out=post_ps, lhsT=bsb[0:64, :],
                                         rhs=Wl_t, start=True, stop=True)
                        dpe = wp.tile([P, 1], fp, tag="dpe")
                        nc.vector.tensor_scalar_add(out=dpe[:], in0=denc_ps,
                                                    scalar1=1e-16)
                        rec = wp.tile([P, 1], fp, tag="rec")
                        nc.vector.reciprocal(out=rec[:], in_=dpe[:])
                        sc = wp.tile([P, c], fp, tag="sc")
                        nc.vector.tensor_scalar(out=sc[:], in0=post_ps,
                                                scalar1=rec[:, 0:1],
                                                scalar2=None, op0=ALU.mult)
                        outt = wp.tile([P, c], fp, tag="outt")
                        nc.vector.tensor_tensor(out=outt[:], in0=sc[:],
                                                in1=bias_t[:], op=ALU.add)
                        nc.scalar.dma_start(
                            out=dd["out"][ds((2 * pv + b) * P, P), :],
                            in_=outt[:])

                    do_trans(0)
                    do_m(0)
                    do_attn(0)
                    do_trans(1)
                    do_scatter(0)
                    do_m(1)
                    do_attn(1)
                    do_scatter(1)

                bodies.append(pair_body)

            if unroll:
                for pv in range(npairs):
                    for body in bodies:
                        body(pv)
            else:
                CHUNK = 12
                for s0 in range(0, npairs, CHUNK):
                    with tc.For_i(s0, min(s0 + CHUNK, npairs), 1,
                                  staggered_reset=True) as pv:
                        for body in bodies:
                            body(pv)

    nc.compile()
    return nc, dirs


def kernel(x0, x1, edge_index, edge_attr,
           Wl_b, Wr_b, We_b, att_b, b_b,
           Wl_f, Wr_f, We_f, att_f, b_f):
    x0 = np.asarray(x0, np.float32)
    x1 = np.asarray(x1, np.float32)
    edge_attr = np.asarray(edge_attr, np.float32)
    ei = np.asarray(edge_index)
    src, dst = ei[0].astype(np.int64), ei[1].astype(np.int64)

    N, d = x0.shape
    de = edge_attr.shape[1]
    c = np.asarray(Wl_b).shape[1]
    qsize = _ceil_div(N, NQ)
    assert qsize <= 32767

    cores_b, npc, npc_pad, nblk = _prep_direction(x1, src, dst, edge_attr, NCORES)
    cores_f, _, _, _ = _prep_direction(x0, dst, src, edge_attr, NCORES)

    tq = _ceil_div(max(_max_run(cores_b, qsize, nblk),
                       _max_run(cores_f, qsize, nblk)), P)

    lay_b = _layout_direction(cores_b, nblk, tq, qsize, de)
    lay_f = _layout_direction(cores_f, nblk, tq, qsize, de)

    nc, dirs = _build_program(nblk, tq, npc_pad, qsize, de, d, c)

    def tabs(x):
        t = []
        for q in range(NQ):
            xx = x[q * qsize:(q + 1) * qsize]
            rows = np.zeros((qsize, d), np.float32)
            rows[:xx.shape[0]] = xx
            t.append(rows)
        return t

    x0t, x1t = tabs(x0), tabs(x1)
    cstb = np.zeros((P, 771), np.float32)
    cstb[:, 0:P] = np.arange(P, dtype=np.float32)[None, :]      # iota
    cstb[:, P:2 * P] = np.eye(P, dtype=np.float32)              # ident
    cstb[:, 2 * P:2 * P + 1] = 1.0                              # ones
    for di, (Wl, Wr, We, att) in enumerate(
            ((Wl_b, Wr_b, We_b, att_b), (Wl_f, Wr_f, We_f, att_f))):
        o = 257 + di * 257
        Wl = np.asarray(Wl, np.float32)
        cstb[0:d, o:o + c] = Wl
        cstb[:, o + c:o + 2 * c] = np.tile(Wl, (2, 1))
        cstb[0:d, o + 2 * c:o + 3 * c] = np.asarray(Wr, np.float32)
        cstb[0:de, o + 3 * c:o + 4 * c] = np.asarray(We, np.float32)
        cstb[:, o + 4 * c:o + 4 * c + 1] = np.tile(
            np.asarray(att, np.float32).reshape(c, 1), (2, 1))
    cstb = cstb.astype(bfloat16)
    cstf = np.zeros((P, 129), np.float32)
    cstf[:, 0:1] = np.arange(P, dtype=np.float32)[:, None]      # iotac
    cstf[:, 1:1 + c] = np.asarray(b_b, np.float32)[None, :]
    cstf[:, 1 + c:1 + 2 * c] = np.asarray(b_f, np.float32)[None, :]

    def xdT_shard(xd, k):
        lo = k * npc
        sh = xd[lo:min(lo + npc, N)]
        pad = np.zeros((npc_pad, d), np.float32)
        pad[:sh.shape[0]] = sh
        return np.ascontiguousarray(pad.T).astype(bfloat16)

    in_maps = []
    for k in range(NCORES):
        (idxw_b, doffC_b, eaT_b, doffR_b) = lay_b[k]
        (idxw_f, doffC_f, eaT_f, doffR_f) = lay_f[k]
        m = {
            "cstb": cstb, "cstf": cstf,
            "idxw_b": idxw_b, "doffC_b": doffC_b, "eaT_b": eaT_b,
            "doffR_b": doffR_b,
            "idxw_f": idxw_f, "doffC_f": doffC_f, "eaT_f": eaT_f,
            "doffR_f": doffR_f,
            "xdT_b": xdT_shard(x1, k), "xdT_f": xdT_shard(x0, k),
        }
        for q in range(NQ):
            m[f"tab{q}_b"] = x0t[q]
            m[f"tab{q}_f"] = x1t[q]
        in_maps.append(m)

    kernel.last_tq, kernel.last_nblk = tq, nblk
    kernel.last_npc_pad, kernel.last_qsize = npc_pad, qsize
    res = run_bass_kernel_spmd(nc, in_maps, list(range(NCORES)))

    out_b = np.concatenate([res.results[k]["out_b"][:npc]
                            for k in range(NCORES)])[:N]
    out_f = np.concatenate([res.results[k]["out_f"][:npc]
                            for k in range(NCORES)])[:N]
    return (out_b, out_f)


# revision 24
# speedup vs baseline: 5.8272x; 1.0416x over previous
"""Bidirectional GATv2Conv (heads=1) on 8 Trainium2 NeuronCores.

Strategy (edge-parallel, dst-sharded -- no collectives):
- dst nodes range-sharded across 8 cores; each core owns every edge whose
  aggregation target is in its range, so segment-softmax stats stay local.
- Edges sorted by (block-pair, src-quarter, block); each (pair, quarter,
  block) run padded to tq tiles of 128 edges -> identical SPMD program.
- All PE/DVE data in bf16 (tolerance 2e-2 >> bf16 error):
    * node tables hold 128-wide rows [x | 1 | 0...]; plain dma_gather
      (elem 256B) yields G [128e, 128] whose col 64 is the ones column
      used to fold the softmax denominator into the scatter matmul.
    * gT via paired PE transposes packed 4-per-PSUM-bank, single
      [128,512] activation-copy escape (8 tiles per escape).
    * m computed c-major in packed PSUM banks (2 tiles per 128
      partitions); single Lrelu(alpha=0.2) escape per bank.
    * xr[dst] delivered via slot-major indicator: dstoff replicated
      across partitions by a stride-0 DMA (uint8), one DVE is_equal per
      512 cols; xr itself precomputed per-core into persistent SBUF.
    * logits batched per block into one PSUM tile; one exp per block.
    * scatter: indw = (iota==dstoff)*ex fused DVE op per tile; numerator
      and denominator accumulate via one [65,128] matmul per tile.
- out = (num/den) @ Wl + bias; Wl applied after aggregation
  (sum_e alpha_e * (x@Wl) == (sum_e alpha_e x) @ Wl).
"""

import numpy as np
from ml_dtypes import bfloat16

import concourse.bass as bass
import concourse.bacc as bacc
import concourse.mybir as mybir
import concourse.tile as tile
from concourse.bass import ds, AP
from concourse.bass_utils import run_bass_kernel_spmd

P = 128
NCORES = 8
NQ = 4           # src-table quarters (int16 idx limit: 32767 >= 25000)


def _ceil_div(a, b):
    return (a + b - 1) // b


def _prep_direction(x_dst, src, dst, ea, n_cores):
    """Per-core edge bucketing (before padding, which needs global TQ)."""
    N = x_dst.shape[0]
    npc = _ceil_div(N, n_cores)
    npc_pad = _ceil_div(npc, P) * P
    nblk = npc_pad // P
    cores = []
    for k in range(n_cores):
        lo = k * npc
        hi = min(lo + npc, N)
        sel = (dst >= lo) & (dst < hi)
        cores.append((src[sel], dst[sel] - lo, ea[sel]))
    return cores, npc, npc_pad, nblk


def _max_run(cores, qsize, nblk):
    m = 0
    for (e_src, e_dst, e_ea) in cores:
        blk = e_dst >> 7
        qua = e_src // qsize
        key = blk * NQ + qua
        cnt = np.bincount(key, minlength=nblk * NQ)
        m = max(m, int(cnt.max()))
    return m


def _layout_direction(cores, nblk, tq, qsize, de):
    """Build padded per-core device arrays (slot order: pair,quarter,block)."""
    npairs = nblk // 2
    run = tq * P
    slots_pair = NQ * 2 * run
    total = npairs * slots_pair
    nrun = npairs * NQ * 2
    out = []
    for (e_src, e_dst, e_ea) in cores:
        blk = e_dst >> 7
        qua = e_src // qsize
        runid = ((blk >> 1) * NQ + qua) * 2 + (blk & 1)
        order = np.argsort(runid, kind="stable")
        s_src = e_src[order]
        s_dst = e_dst[order]
        s_ea = e_ea[order]
        s_run = runid[order]
        s_loc = (s_src - (s_src // qsize) * qsize).astype(np.int16)
        s_off = (s_dst & 127).astype(np.uint8)

        idx_all = np.zeros(total, np.int16)
        doff_u8 = np.full(total, 255, np.uint8)
        doff_f = np.full(total, -1.0, np.float32)
        ea_all = np.zeros((total, de), np.float32)
        starts = np.searchsorted(s_run, np.arange(nrun + 1))
        for r in range(nrun):
            s0, s1 = int(starts[r]), int(starts[r + 1])
            cnt = s1 - s0
            assert cnt <= run, f"run {r} has {cnt} > {run} edges"
            base = r * run
            idx_all[base:base + cnt] = s_loc[s0:s1]
            doff_u8[base:base + cnt] = s_off[s0:s1]
            doff_f[base:base + cnt] = s_off[s0:s1]
            ea_all[base:base + cnt] = s_ea[s0:s1]

        idxw = np.tile(idx_all.reshape(-1, 16).T, (8, 1)).copy()
        doffC = doff_f.reshape(-1, P).T.copy()               # [128, total/128]
        eaT = np.ascontiguousarray(ea_all.T).astype(bfloat16)  # [de, total]
        doffR = doff_u8.reshape(1, total)
        out.append((idxw, doffC, eaT, doffR))
    return out


def _build_program(nblk, tq, npc_pad, qsize, de, d, c, unroll=False):
    assert nblk % 2 == 0
    npairs = nblk // 2
    ntile = NQ * tq
    slots_pair = NQ * 2 * tq * P
    fp = mybir.dt.float32
    bf = mybir.dt.bfloat16
    u8 = mybir.dt.uint8
    i16 = mybir.dt.int16
    AF = mybir.ActivationFunctionType
    ALU = mybir.AluOpType
    nc = bacc.Bacc("TRN2")

    def dram(name, shape, dt=fp, out=False):
        return nc.declare_dram_parameter(name, list(shape), dt, isOutput=out)

    dirs = {}
    for dn in ("b", "f"):
        dirs[dn] = dict(
            q_tabs=[dram(f"tab{q}_{dn}", [qsize, d]) for q in range(NQ)],
            idxw=dram(f"idxw_{dn}", [P, npairs * slots_pair // 16], i16),
            doffC=dram(f"doffC_{dn}", [P, npairs * 2 * ntile]),
            doffR=dram(f"doffR_{dn}", [1, npairs * slots_pair], u8),
            eaT=dram(f"eaT_{dn}", [de, npairs * slots_pair], bf),
            xdT=dram(f"xdT_{dn}", [d, npc_pad], bf),
            out=dram(f"out_{dn}", [npc_pad, c], out=True),
        )
    cstb_d = dram("cstb", [P, 771], bf)   # packed bf16 consts
    cstf_d = dram("cstf", [P, 129])       # packed fp32 consts

    # transpose slot list per block: (quarter, first tile, tiles in pair)
    tlist = []
    for q in range(NQ):
        for i0 in range(0, tq, 2):
            tlist.append((q, i0, min(2, tq - i0)))
    ntrans = len(tlist)
    tpq = _ceil_div(tq, 2)                      # transposes per quarter
    nbank_t = _ceil_div(ntrans, 4)              # gT psum banks per block
    nbank_m = _ceil_div(ntile, 8)               # m psum banks per block

    def t_slot(q, i):
        """tile (q, i) -> (gT bank, col, partition base)"""
        j = q * tpq + i // 2
        return j // 4, (j % 4) * P, (i % 2) * 64

    def m_slot(t):
        """tile t -> (m bank, partition base, col)"""
        return t // 8, (t % 2) * 64, ((t // 2) % 4) * P

    with tile.TileContext(nc) as tc:
        with tc.tile_pool(name="const", bufs=1) as cp, \
             tc.tile_pool(name="xr", bufs=1) as xp, \
             tc.tile_pool(name="load", bufs=2) as lp, \
             tc.tile_pool(name="gt", bufs=3) as gp, \
             tc.tile_pool(name="m8", bufs=3) as mp8, \
             tc.tile_pool(name="ind", bufs=20) as ip, \
             tc.tile_pool(name="work", bufs=4) as wp, \
             tc.tile_pool(name="indw", bufs=24) as iw, \
             tc.tile_pool(name="ps_t", bufs=3, space="PSUM") as ps_t, \
             tc.tile_pool(name="ps_m", bufs=3, space="PSUM") as ps_m, \
             tc.tile_pool(name="ps_s", bufs=2, space="PSUM") as ps_s:

            cstb_t = cp.tile([P, 771], bf)
            nc.scalar.dma_start(out=cstb_t[:], in_=cstb_d[:])
            cstf_t = cp.tile([P, 129], fp)
            nc.scalar.dma_start(out=cstf_t[:], in_=cstf_d[:])
            iota_t = cstb_t[:, 0:P]
            ident_t = cstb_t[:, P:2 * P]
            ones_t = cstb_t[:, 2 * P:2 * P + 1]
            iotac_t = cstf_t[:, 0:1]

            bodies = []
            pending_sc = []
            for di, dn in enumerate(("b", "f")):
                dd = dirs[dn]
                o = 257 + di * 257
                Wl_t = cstb_t[0:d, o:o + c]
                Wl2_t = cstb_t[:, o + c:o + 2 * c]
                Wr_t = cstb_t[0:d, o + 2 * c:o + 3 * c]
                We_t = cstb_t[0:de, o + 3 * c:o + 4 * c]
                att2_t = cstb_t[:, o + 4 * c:o + 4 * c + 1]
                bias_t = cstf_t[:, 1 + di * c:1 + (di + 1) * c]

                # ---- prologue: xr = x_dst @ Wr for all own blocks ----
                xr_sb = xp.tile([P, nblk * c], bf, tag=f"xr{dn}")
                for g in range(_ceil_div(nblk, 8)):
                    nb = min(8, nblk - 8 * g)
                    xd_t = lp.tile([d, 8 * P], bf, tag="xd")
                    nc.scalar.dma_start(out=xd_t[:, 0:nb * P],
                                        in_=dd["xdT"][:, ds(g * 8 * P, nb * P)])
                    pro_ps = ps_m.tile([P, 512], fp, tag="mb")
                    for j in range(nb):
                        nc.tensor.matmul(out=pro_ps[:, j * c:(j + 1) * c],
                                         lhsT=xd_t[:, j * P:(j + 1) * P],
                                         rhs=Wr_t, start=True, stop=True)
                    nc.scalar.activation(out=xr_sb[:, ds(g * 8 * c, nb * c)],
                                         in_=pro_ps[:, 0:nb * c], func=AF.Copy)

                def pair_body(pv, dd=dd, Wl_t=Wl_t, Wl2_t=Wl2_t,
                              We_t=We_t, att2_t=att2_t, bias_t=bias_t,
                              xr_sb=xr_sb):
                    base = pv * slots_pair
                    idxw_t = lp.tile([P, slots_pair // 16], i16, tag="idxw")
                    nc.sync.dma_start(
                        out=idxw_t[:],
                        in_=dd["idxw"][:, ds(pv * (slots_pair // 16),
                                             slots_pair // 16)])
                    ea_t = lp.tile([de, slots_pair], bf, tag="ea")
                    nc.sync.dma_start(out=ea_t[:],
                                      in_=dd["eaT"][:, ds(base, slots_pair)])
                    dC_t = lp.tile([P, 2 * ntile], fp, tag="dC")
                    nc.sync.dma_start(out=dC_t[:],
                                      in_=dd["doffC"][:, ds(pv * 2 * ntile,
                                                            2 * ntile)])
                    dR_t = lp.tile([P, slots_pair], u8, tag="dR")
                    src = dd["doffR"][0:1, ds(base, slots_pair)]
                    rep = AP(src.tensor, src.offset, [[0, P]] + src.ap[1:])
                    nc.sync.dma_start(out=dR_t[:], in_=rep)

                    xr_pair = lp.tile([P, 2 * c], bf, tag="xrp")
                    nc.sync.dma_start(out=xr_pair[:],
                                      in_=xr_sb[:, ds(pv * 2 * c, 2 * c)])
                    G_ts = []
                    for q in range(NQ):
                        G32 = lp.tile([P, 2 * tq, d], fp, tag=f"G32_{q}")
                        for b in (0, 1):
                            run = q * 2 + b
                            nc.gpsimd.dma_gather(
                                out_ap=G32[:, b * tq:(b + 1) * tq, :],
                                in_ap=dd["q_tabs"][q][:],
                                idxs_ap=idxw_t[:, run * (tq * 8):
                                               (run + 1) * (tq * 8)],
                                num_idxs=tq * P,
                                num_idxs_reg=tq * P,
                                elem_size=d,
                            )
                        G = lp.tile([P, 2 * tq, c + 1], bf, tag=f"G{q}")
                        nc.vector.memset(G[:, :, c:c + 1], 1.0)
                        nc.vector.tensor_scalar_mul(out=G[:, :, 0:c],
                                                    in0=G32[:, :, :],
                                                    scalar1=1.0)
                        G_ts.append(G)

                    # slot-major indicator, one tile per (quarter, block)
                    # run, block-0 runs emitted first so m(0) is fed early
                    run_sz = tq * P
                    indT_ch = [None] * (2 * NQ)
                    for run in [q * 2 + b for b in (0, 1) for q in range(NQ)]:
                        it = ip.tile([P, run_sz], bf, tag="indT", name="indT")
                        nc.vector.tensor_scalar(
                            out=it[:],
                            in0=dR_t[:, ds(run * run_sz, run_sz)],
                            scalar1=iotac_t, scalar2=None,
                            op0=ALU.is_equal)
                        indT_ch[run] = it

                    # skewed per-block phases: block 1's transposes are
                    # emitted between block 0's attention and scatter so PE
                    # stays busy while exp/indw cook on ACT/DVE.
                    gT8s = {}
                    m8s = {}
                    ex_ts = {}
                    S_ts = {}
                    indw_ts = {}

                    def do_trans(b):
                        for k in range(nbank_t):
                            tbank = ps_t.tile([P, 512], bf, tag="tb", name="tb")
                            for j, (q, i0, w) in enumerate(tlist):
                                if j // 4 != k:
                                    continue
                                for u in range(w):
                                    nc.tensor.transpose(
                                        out=tbank[u * 64:u * 64 + 64,
                                                  ds((j % 4) * P, P)],
                                        in_=G_ts[q][:, b * tq + i0 + u, 0:64],
                                        identity=ident_t)
                            g8 = gp.tile([P, 512], bf, tag="g8")
                            nc.scalar.activation(out=g8[:], in_=tbank[:],
                                                 func=AF.Copy)
                            gT8s[(b, k)] = g8

                    def do_m(b):
                        mbanks = [ps_m.tile([P, 512], fp, tag="mb", name="mb")
                                  for _ in range(nbank_m)]
                        for t in range(ntile):
                            q, i = t // tq, t % tq
                            tb, tcol, tpb = t_slot(q, i)
                            mb, mpb, mcol = m_slot(t)
                            dst = mbanks[mb][mpb:mpb + c, ds(mcol, P)]
                            rhs_g = gT8s[(b, tb)][tpb:tpb + 64, ds(tcol, P)]
                            scol = ((q * 2 + b) * tq + i) * P
                            nc.tensor.matmul(out=dst,
                                             lhsT=Wl2_t[tpb:tpb + d, :],
                                             rhs=rhs_g,
                                             start=True, stop=False)
                            nc.tensor.matmul(out=dst, lhsT=We_t,
                                             rhs=ea_t[:, ds(scol, P)],
                                             start=False, stop=False)
                            nc.tensor.matmul(out=dst,
                                             lhsT=xr_pair[:, b * c:(b + 1) * c],
                                             rhs=indT_ch[q * 2 + b]
                                             [:, ds(i * P, P)],
                                             start=False, stop=True)
                            if (t + 1) % 8 == 0 or t == ntile - 1:
                                k = t // 8
                                n_t = min(8, ntile - 8 * k)
                                cols = _ceil_div(n_t, 2) * P
                                m8 = mp8.tile([P, 512], bf, tag="m8")
                                nc.scalar.activation(out=m8[:, 0:cols],
                                                     in_=mbanks[k][:, 0:cols],
                                                     func=AF.Prelu, alpha=0.2)
                                m8s[(b, k)] = m8

                    def do_attn(b):
                        S = ps_s.tile([P, 512], fp, tag="sb", name="sb")
                        S_ts[b] = S
                        lg_ps = S[:, 0:ntile]
                        for t in range(ntile):
                            mb, mpb, mcol = m_slot(t)
                            nc.tensor.matmul(
                                out=lg_ps[:, t:t + 1],
                                lhsT=m8s[(b, mb)][mpb:mpb + c, ds(mcol, P)],
                                rhs=att2_t[mpb:mpb + c, :],
                                start=True, stop=True)
                        ex_t = wp.tile([P, ntile], fp, tag="ex")
                        nc.scalar.activation(out=ex_t[:], in_=lg_ps[:],
                                             func=AF.Exp)
                        ex_ts[b] = ex_t
                        for t in range(ntile):
                            q, i = t // tq, t % tq
                            indw = iw.tile([P, P], bf, tag="iw")
                            nc.vector.tensor_scalar(
                                out=indw[:], in0=iota_t,
                                scalar1=dC_t[:, (q * 2 + b) * tq + i:
                                             (q * 2 + b) * tq + i + 1],
                                scalar2=ex_t[:, t:t + 1],
                                op0=ALU.is_equal, op1=ALU.mult)
                            indw_ts[(b, t)] = indw

                    def do_scatter(b):
                        S = S_ts[b]
                        blk_ps = S[0:65, 256:384]
                        for t in range(ntile):
                            q, i = t // tq, t % tq
                            nc.tensor.matmul(
                                out=blk_ps,
                                lhsT=G_ts[q][:, b * tq + i, 0:c + 1],
                                rhs=indw_ts[(b, t)][:],
                                start=(t == 0), stop=(t == ntile - 1))
                        bsb = wp.tile([65, P], bf, tag="bsb")
                        nc.scalar.activation(out=bsb[:], in_=blk_ps,
                                             func=AF.Copy)
                        denc_ps = S[:, 384:385]
                        nc.tensor.matmul(out=denc_ps, lhsT=bsb[64:65, :],
                                         rhs=ones_t[64:65, 0:1],
                                         start=True, stop=True)
                        post_ps = S[:, 448:448 + c]
                        nc.tensor.matmul(out=post_ps, lhsT=bsb[0:64, :],
                                         rhs=Wl_t, start=True, stop=True)
                        dpe = wp.tile([P, 1], fp, tag="dpe")
                        nc.vector.tensor_scalar_add(out=dpe[:], in0=denc_ps,
                                                    scalar1=1e-16)
                        rec = wp.tile([P, 1], fp, tag="rec")
                        nc.vector.reciprocal(out=rec[:], in_=dpe[:])
                        sc = wp.tile([P, c], fp, tag="sc")
                        nc.vector.tensor_scalar(out=sc[:], in0=post_ps,
                                                scalar1=rec[:, 0:1],
                                                scalar2=None, op0=ALU.mult)
                        outt = wp.tile([P, c], fp, tag="outt")
                        nc.vector.tensor_tensor(out=outt[:], in0=sc[:],
                                                in1=bias_t[:], op=ALU.add)
                        nc.scalar.dma_start(
                            out=dd["out"][ds((2 * pv + b) * P, P), :],
                            in_=outt[:])

                    do_trans(0)
                    while pending_sc:
                        pending_sc.pop(0)()
                    do_m(0)
                    do_attn(0)
                    do_trans(1)
                    do_scatter(0)
                    do_m(1)
                    do_attn(1)
                    pending_sc.append(lambda: do_scatter(1))

                bodies.append(pair_body)

            def run_iter(pv):
                for body in bodies:
                    body(pv)
                while pending_sc:
                    pending_sc.pop(0)()

            if unroll:
                for pv in range(npairs):
                    run_iter(pv)
            else:
                CHUNK = 12
                for s0 in range(0, npairs, CHUNK):
                    with tc.For_i(s0, min(s0 + CHUNK, npairs), 1,
                                  staggered_reset=True) as pv:
                        run_iter(pv)

    nc.compile()
    return nc, dirs


def kernel(x0, x1, edge_index, edge_attr,
           Wl_b, Wr_b, We_b, att_b, b_b,
           Wl_f, Wr_f, We_f, att_f, b_f):
    x0 = np.asarray(x0, np.float32)
    x1 = np.asarray(x1, np.float32)
    edge_attr = np.asarray(edge_attr, np.float32)
    ei = np.asarray(edge_index)
    src, dst = ei[0].astype(np.int64), ei[1].astype(np.int64)

    N, d = x0.shape
    de = edge_attr.shape[1]
    c = np.asarray(Wl_b).shape[1]
    qsize = _ceil_div(N, NQ)
    assert qsize <= 32767

    cores_b, npc, npc_pad, nblk = _prep_direction(x1, src, dst, edge_attr, NCORES)
    cores_f, _, _, _ = _prep_direction(x0, dst, src, edge_attr, NCORES)

    tq = _ceil_div(max(_max_run(cores_b, qsize, nblk),
                       _max_run(cores_f, qsize, nblk)), P)

    lay_b = _layout_direction(cores_b, nblk, tq, qsize, de)
    lay_f = _layout_direction(cores_f, nblk, tq, qsize, de)

    nc, dirs = _build_program(nblk, tq, npc_pad, qsize, de, d, c)

    def tabs(x):
        t = []
        for q in range(NQ):
            xx = x[q * qsize:(q + 1) * qsize]
            rows = np.zeros((qsize, d), np.float32)
            rows[:xx.shape[0]] = xx
            t.append(rows)
        return t

    x0t, x1t = tabs(x0), tabs(x1)
    cstb = np.zeros((P, 771), np.float32)
    cstb[:, 0:P] = np.arange(P, dtype=np.float32)[None, :]      # iota
    cstb[:, P:2 * P] = np.eye(P, dtype=np.float32)              # ident
    cstb[:, 2 * P:2 * P + 1] = 1.0                              # ones
    for di, (Wl, Wr, We, att) in enumerate(
            ((Wl_b, Wr_b, We_b, att_b), (Wl_f, Wr_f, We_f, att_f))):
        o = 257 + di * 257
        Wl = np.asarray(Wl, np.float32)
        cstb[0:d, o:o + c] = Wl
        cstb[:, o + c:o + 2 * c] = np.tile(Wl, (2, 1))
        cstb[0:d, o + 2 * c:o + 3 * c] = np.asarray(Wr, np.float32)
        cstb[0:de, o + 3 * c:o + 4 * c] = np.asarray(We, np.float32)
        cstb[:, o + 4 * c:o + 4 * c + 1] = np.tile(
            np.asarray(att, np.float32).reshape(c, 1), (2, 1))
    cstb = cstb.astype(bfloat16)
    cstf = np.zeros((P, 129), np.float32)
    cstf[:, 0:1] = np.arange(P, dtype=np.float32)[:, None]      # iotac
    cstf[:, 1:1 + c] = np.asarray(b_b, np.float32)[None, :]
    cstf[:, 1 + c:1 + 2 * c] = np.asarray(b_f, np.float32)[None, :]

    def xdT_shard(xd, k):
        lo = k * npc
        sh = xd[lo:min(lo + npc, N)]
        pad = np.zeros((npc_pad, d), np.float32)
        pad[:sh.shape[0]] = sh
        return np.ascontiguousarray(pad.T).astype(bfloat16)

    in_maps = []
    for k in range(NCORES):
        (idxw_b, doffC_b, eaT_b, doffR_b) = lay_b[k]
        (idxw_f, doffC_f, eaT_f, doffR_f) = lay_f[k]
        m = {
            "cstb": cstb, "cstf": cstf,
            "idxw_b": idxw_b, "doffC_b": doffC_b, "eaT_b": eaT_b,
            "doffR_b": doffR_b,
            "idxw_f": idxw_f, "doffC_f": doffC_f, "eaT_f": eaT_f,
            "doffR_f": doffR_f,
            "xdT_b": xdT_shard(x1, k), "xdT_f": xdT_shard(x0, k),
        }
        for q in range(NQ):
            m[f"tab{q}_b"] = x0t[q]
            m[f"tab{q}_f"] = x1t[q]
        in_maps.append(m)

    kernel.last_tq, kernel.last_nblk = tq, nblk
    kernel.last_npc_pad, kernel.last_qsize = npc_pad, qsize
    res = run_bass_kernel_spmd(nc, in_maps, list(range(NCORES)))

    out_b = np.concatenate([res.results[k]["out_b"][:npc]
                            for k in range(NCORES)])[:N]
    out_f = np.concatenate([res.results[k]["out_f"][:npc]
                            for k in range(NCORES)])[:N]
    return (out_b, out_f)


# revision 25
# speedup vs baseline: 5.9299x; 1.0176x over previous
"""Bidirectional GATv2Conv (heads=1) on 8 Trainium2 NeuronCores.

Strategy (edge-parallel, dst-sharded -- no collectives):
- dst nodes range-sharded across 8 cores; each core owns every edge whose
  aggregation target is in its range, so segment-softmax stats stay local.
- Edges sorted by (block-pair, src-quarter, block); each (pair, quarter,
  block) run padded to tq tiles of 128 edges -> identical SPMD program.
- All PE/DVE data in bf16 (tolerance 2e-2 >> bf16 error):
    * node tables hold 128-wide rows [x | 1 | 0...]; plain dma_gather
      (elem 256B) yields G [128e, 128] whose col 64 is the ones column
      used to fold the softmax denominator into the scatter matmul.
    * gT via paired PE transposes packed 4-per-PSUM-bank, single
      [128,512] activation-copy escape (8 tiles per escape).
    * m computed c-major in packed PSUM banks (2 tiles per 128
      partitions); single Lrelu(alpha=0.2) escape per bank.
    * xr[dst] delivered via slot-major indicator: dstoff replicated
      across partitions by a stride-0 DMA (uint8), one DVE is_equal per
      512 cols; xr itself precomputed per-core into persistent SBUF.
    * logits batched per block into one PSUM tile; one exp per block.
    * scatter: indw = (iota==dstoff)*ex fused DVE op per tile; numerator
      and denominator accumulate via one [65,128] matmul per tile.
- out = (num/den) @ Wl + bias; Wl applied after aggregation
  (sum_e alpha_e * (x@Wl) == (sum_e alpha_e x) @ Wl).
"""

import numpy as np
from ml_dtypes import bfloat16

import concourse.bass as bass
import concourse.bacc as bacc
import concourse.mybir as mybir
import concourse.tile as tile
from concourse.bass import ds, AP
from concourse.bass_utils import run_bass_kernel_spmd

P = 128
NCORES = 8
NQ = 4           # src-table quarters (int16 idx limit: 32767 >= 25000)


def _ceil_div(a, b):
    return (a + b - 1) // b


def _prep_direction(x_dst, src, dst, ea, n_cores):
    """Per-core edge bucketing (before padding, which needs global TQ)."""
    N = x_dst.shape[0]
    npc = _ceil_div(N, n_cores)
    npc_pad = _ceil_div(npc, P) * P
    nblk = npc_pad // P
    cores = []
    for k in range(n_cores):
        lo = k * npc
        hi = min(lo + npc, N)
        sel = (dst >= lo) & (dst < hi)
        cores.append((src[sel], dst[sel] - lo, ea[sel]))
    return cores, npc, npc_pad, nblk


def _max_run(cores, qsize, nblk):
    m = 0
    for (e_src, e_dst, e_ea) in cores:
        blk = e_dst >> 7
        qua = e_src // qsize
        key = blk * NQ + qua
        cnt = np.bincount(key, minlength=nblk * NQ)
        m = max(m, int(cnt.max()))
    return m


def _layout_direction(cores, nblk, tq, qsize, de):
    """Build padded per-core device arrays (slot order: pair,quarter,block)."""
    npairs = nblk // 2
    run = tq * P
    slots_pair = NQ * 2 * run
    total = npairs * slots_pair
    nrun = npairs * NQ * 2
    out = []
    for (e_src, e_dst, e_ea) in cores:
        blk = e_dst >> 7
        qua = e_src // qsize
        runid = ((blk >> 1) * NQ + qua) * 2 + (blk & 1)
        order = np.argsort(runid, kind="stable")
        s_src = e_src[order]
        s_dst = e_dst[order]
        s_ea = e_ea[order]
        s_run = runid[order]
        s_loc = (s_src - (s_src // qsize) * qsize).astype(np.int16)
        s_off = (s_dst & 127).astype(np.uint8)

        idx_all = np.zeros(total, np.int16)
        doff_u8 = np.full(total, 255, np.uint8)
        doff_f = np.full(total, -1.0, np.float32)
        ea_all = np.zeros((total, de), np.float32)
        starts = np.searchsorted(s_run, np.arange(nrun + 1))
        for r in range(nrun):
            s0, s1 = int(starts[r]), int(starts[r + 1])
            cnt = s1 - s0
            assert cnt <= run, f"run {r} has {cnt} > {run} edges"
            base = r * run
            idx_all[base:base + cnt] = s_loc[s0:s1]
            doff_u8[base:base + cnt] = s_off[s0:s1]
            doff_f[base:base + cnt] = s_off[s0:s1]
            ea_all[base:base + cnt] = s_ea[s0:s1]

        idxw = np.tile(idx_all.reshape(-1, 16).T, (8, 1)).copy()
        doffC = doff_f.reshape(-1, P).T.copy()               # [128, total/128]
        eaT = np.ascontiguousarray(ea_all.T).astype(bfloat16)  # [de, total]
        doffR = doff_u8.reshape(1, total)
        out.append((idxw, doffC, eaT, doffR))
    return out


def _build_program(nblk, tq, npc_pad, qsize, de, d, c, unroll=False):
    assert nblk % 2 == 0
    npairs = nblk // 2
    ntile = NQ * tq
    slots_pair = NQ * 2 * tq * P
    fp = mybir.dt.float32
    bf = mybir.dt.bfloat16
    u8 = mybir.dt.uint8
    i16 = mybir.dt.int16
    AF = mybir.ActivationFunctionType
    ALU = mybir.AluOpType
    nc = bacc.Bacc("TRN2")

    def dram(name, shape, dt=fp, out=False):
        return nc.declare_dram_parameter(name, list(shape), dt, isOutput=out)

    dirs = {}
    for dn in ("b", "f"):
        dirs[dn] = dict(
            q_tabs=[dram(f"tab{q}_{dn}", [qsize, d]) for q in range(NQ)],
            idxw=dram(f"idxw_{dn}", [P, npairs * slots_pair // 16], i16),
            doffC=dram(f"doffC_{dn}", [P, npairs * 2 * ntile]),
            doffR=dram(f"doffR_{dn}", [1, npairs * slots_pair], u8),
            eaT=dram(f"eaT_{dn}", [de, npairs * slots_pair], bf),
            xdT=dram(f"xdT_{dn}", [d, npc_pad], bf),
            out=dram(f"out_{dn}", [npc_pad, c], out=True),
        )
    cstb_d = dram("cstb", [P, 771], bf)   # packed bf16 consts
    cstf_d = dram("cstf", [P, 129])       # packed fp32 consts

    # transpose slot list per block: (quarter, first tile, tiles in pair)
    tlist = []
    for q in range(NQ):
        for i0 in range(0, tq, 2):
            tlist.append((q, i0, min(2, tq - i0)))
    ntrans = len(tlist)
    tpq = _ceil_div(tq, 2)                      # transposes per quarter
    nbank_t = _ceil_div(ntrans, 4)              # gT psum banks per block
    nbank_m = _ceil_div(ntile, 8)               # m psum banks per block

    def t_slot(q, i):
        """tile (q, i) -> (gT bank, col, partition base)"""
        j = q * tpq + i // 2
        return j // 4, (j % 4) * P, (i % 2) * 64

    def m_slot(t):
        """tile t -> (m bank, partition base, col)"""
        return t // 8, (t % 2) * 64, ((t // 2) % 4) * P

    with tile.TileContext(nc) as tc:
        with tc.tile_pool(name="const", bufs=1) as cp, \
             tc.tile_pool(name="xr", bufs=1) as xp, \
             tc.tile_pool(name="load", bufs=2) as lp, \
             tc.tile_pool(name="gt", bufs=3) as gp, \
             tc.tile_pool(name="m8", bufs=3) as mp8, \
             tc.tile_pool(name="ind", bufs=20) as ip, \
             tc.tile_pool(name="work", bufs=4) as wp, \
             tc.tile_pool(name="indw", bufs=24) as iw, \
             tc.tile_pool(name="ps_t", bufs=3, space="PSUM") as ps_t, \
             tc.tile_pool(name="ps_m", bufs=3, space="PSUM") as ps_m, \
             tc.tile_pool(name="ps_s", bufs=2, space="PSUM") as ps_s:

            cstb_t = cp.tile([P, 771], bf)
            nc.sync.dma_start(out=cstb_t[:], in_=cstb_d[:])
            cstf_t = cp.tile([P, 129], fp)
            nc.sync.dma_start(out=cstf_t[:], in_=cstf_d[:])
            iota_t = cstb_t[:, 0:P]
            ident_t = cstb_t[:, P:2 * P]
            ones_t = cstb_t[:, 2 * P:2 * P + 1]
            iotac_t = cstf_t[:, 0:1]

            bodies = []
            pending_sc = []
            for di, dn in enumerate(("b", "f")):
                dd = dirs[dn]
                o = 257 + di * 257
                Wl_t = cstb_t[0:d, o:o + c]
                Wl2_t = cstb_t[:, o + c:o + 2 * c]
                Wr_t = cstb_t[0:d, o + 2 * c:o + 3 * c]
                We_t = cstb_t[0:de, o + 3 * c:o + 4 * c]
                att2_t = cstb_t[:, o + 4 * c:o + 4 * c + 1]
                bias_t = cstf_t[:, 1 + di * c:1 + (di + 1) * c]

                # ---- prologue: xr = x_dst @ Wr for all own blocks ----
                xr_sb = xp.tile([P, nblk * c], bf, tag=f"xr{dn}")
                for g in range(_ceil_div(nblk, 8)):
                    nb = min(8, nblk - 8 * g)
                    xd_t = lp.tile([d, 8 * P], bf, tag="xd")
                    nc.scalar.dma_start(out=xd_t[:, 0:nb * P],
                                        in_=dd["xdT"][:, ds(g * 8 * P, nb * P)])
                    pro_ps = ps_m.tile([P, 512], fp, tag="mb")
                    for j in range(nb):
                        nc.tensor.matmul(out=pro_ps[:, j * c:(j + 1) * c],
                                         lhsT=xd_t[:, j * P:(j + 1) * P],
                                         rhs=Wr_t, start=True, stop=True)
                    nc.scalar.activation(out=xr_sb[:, ds(g * 8 * c, nb * c)],
                                         in_=pro_ps[:, 0:nb * c], func=AF.Copy)

                def pair_body(pv, dd=dd, Wl_t=Wl_t, Wl2_t=Wl2_t,
                              We_t=We_t, att2_t=att2_t, bias_t=bias_t,
                              xr_sb=xr_sb):
                    base = pv * slots_pair
                    idxw_t = lp.tile([P, slots_pair // 16], i16, tag="idxw")
                    nc.sync.dma_start(
                        out=idxw_t[:],
                        in_=dd["idxw"][:, ds(pv * (slots_pair // 16),
                                             slots_pair // 16)])
                    ea_t = lp.tile([de, slots_pair], bf, tag="ea")
                    nc.sync.dma_start(out=ea_t[:],
                                      in_=dd["eaT"][:, ds(base, slots_pair)])
                    dC_t = lp.tile([P, 2 * ntile], fp, tag="dC")
                    nc.sync.dma_start(out=dC_t[:],
                                      in_=dd["doffC"][:, ds(pv * 2 * ntile,
                                                            2 * ntile)])
                    dR_t = lp.tile([P, slots_pair], u8, tag="dR")
                    src = dd["doffR"][0:1, ds(base, slots_pair)]
                    rep = AP(src.tensor, src.offset, [[0, P]] + src.ap[1:])
                    nc.sync.dma_start(out=dR_t[:], in_=rep)

                    xr_pair = lp.tile([P, 2 * c], bf, tag="xrp")
                    nc.sync.dma_start(out=xr_pair[:],
                                      in_=xr_sb[:, ds(pv * 2 * c, 2 * c)])
                    G_ts = []
                    for q in range(NQ):
                        G32 = lp.tile([P, 2 * tq, d], fp, tag=f"G32_{q}")
                        for b in (0, 1):
                            run = q * 2 + b
                            nc.gpsimd.dma_gather(
                                out_ap=G32[:, b * tq:(b + 1) * tq, :],
                                in_ap=dd["q_tabs"][q][:],
                                idxs_ap=idxw_t[:, run * (tq * 8):
                                               (run + 1) * (tq * 8)],
                                num_idxs=tq * P,
                                num_idxs_reg=tq * P,
                                elem_size=d,
                            )
                        G = lp.tile([P, 2 * tq, c + 1], bf, tag=f"G{q}")
                        nc.vector.memset(G[:, :, c:c + 1], 1.0)
                        nc.vector.tensor_scalar_mul(out=G[:, :, 0:c],
                                                    in0=G32[:, :, :],
                                                    scalar1=1.0)
                        G_ts.append(G)

                    # slot-major indicator, one tile per (quarter, block)
                    # run, block-0 runs emitted first so m(0) is fed early
                    run_sz = tq * P
                    indT_ch = [None] * (2 * NQ)
                    for run in [q * 2 + b for b in (0, 1) for q in range(NQ)]:
                        it = ip.tile([P, run_sz], bf, tag="indT", name="indT")
                        nc.vector.tensor_scalar(
                            out=it[:],
                            in0=dR_t[:, ds(run * run_sz, run_sz)],
                            scalar1=iotac_t, scalar2=None,
                            op0=ALU.is_equal)
                        indT_ch[run] = it

                    # skewed per-block phases: block 1's transposes are
                    # emitted between block 0's attention and scatter so PE
                    # stays busy while exp/indw cook on ACT/DVE.
                    gT8s = {}
                    m8s = {}
                    ex_ts = {}
                    S_ts = {}
                    indw_ts = {}

                    def do_trans(b):
                        for k in range(nbank_t):
                            tbank = ps_t.tile([P, 512], bf, tag="tb", name="tb")
                            for j, (q, i0, w) in enumerate(tlist):
                                if j // 4 != k:
                                    continue
                                for u in range(w):
                                    nc.tensor.transpose(
                                        out=tbank[u * 64:u * 64 + 64,
                                                  ds((j % 4) * P, P)],
                                        in_=G_ts[q][:, b * tq + i0 + u, 0:64],
                                        identity=ident_t)
                            g8 = gp.tile([P, 512], bf, tag="g8")
                            if k % 2 == 0:
                                nc.scalar.activation(out=g8[:], in_=tbank[:],
                                                     func=AF.Copy)
                            else:
                                nc.vector.tensor_scalar_mul(out=g8[:],
                                                            in0=tbank[:],
                                                            scalar1=1.0)
                            gT8s[(b, k)] = g8

                    def do_m(b):
                        mbanks = [ps_m.tile([P, 512], fp, tag="mb", name="mb")
                                  for _ in range(nbank_m)]
                        for t in range(ntile):
                            q, i = t // tq, t % tq
                            tb, tcol, tpb = t_slot(q, i)
                            mb, mpb, mcol = m_slot(t)
                            dst = mbanks[mb][mpb:mpb + c, ds(mcol, P)]
                            rhs_g = gT8s[(b, tb)][tpb:tpb + 64, ds(tcol, P)]
                            scol = ((q * 2 + b) * tq + i) * P
                            nc.tensor.matmul(out=dst,
                                             lhsT=Wl2_t[tpb:tpb + d, :],
                                             rhs=rhs_g,
                                             start=True, stop=False)
                            nc.tensor.matmul(out=dst, lhsT=We_t,
                                             rhs=ea_t[:, ds(scol, P)],
                                             start=False, stop=False)
                            nc.tensor.matmul(out=dst,
                                             lhsT=xr_pair[:, b * c:(b + 1) * c],
                                             rhs=indT_ch[q * 2 + b]
                                             [:, ds(i * P, P)],
                                             start=False, stop=True)
                            if (t + 1) % 8 == 0 or t == ntile - 1:
                                k = t // 8
                                n_t = min(8, ntile - 8 * k)
                                cols = _ceil_div(n_t, 2) * P
                                m8 = mp8.tile([P, 512], bf, tag="m8")
                                nc.scalar.activation(out=m8[:, 0:cols],
                                                     in_=mbanks[k][:, 0:cols],
                                                     func=AF.Prelu, alpha=0.2)
                                m8s[(b, k)] = m8

                    def do_attn(b):
                        S = ps_s.tile([P, 512], fp, tag="sb", name="sb")
                        S_ts[b] = S
                        lg_ps = S[:, 0:ntile]
                        for t in range(ntile):
                            mb, mpb, mcol = m_slot(t)
                            nc.tensor.matmul(
                                out=lg_ps[:, t:t + 1],
                                lhsT=m8s[(b, mb)][mpb:mpb + c, ds(mcol, P)],
                                rhs=att2_t[mpb:mpb + c, :],
                                start=True, stop=True)
                        ex_t = wp.tile([P, ntile], fp, tag="ex")
                        nc.scalar.activation(out=ex_t[:], in_=lg_ps[:],
                                             func=AF.Exp)
                        ex_ts[b] = ex_t
                        for t in range(ntile):
                            q, i = t // tq, t % tq
                            indw = iw.tile([P, P], bf, tag="iw")
                            nc.vector.tensor_scalar(
                                out=indw[:], in0=iota_t,
                                scalar1=dC_t[:, (q * 2 + b) * tq + i:
                                             (q * 2 + b) * tq + i + 1],
                                scalar2=ex_t[:, t:t + 1],
                                op0=ALU.is_equal, op1=ALU.mult)
                            indw_ts[(b, t)] = indw

                    def do_scatter(b):
                        S = S_ts[b]
                        blk_ps = S[0:65, 256:384]
                        for t in range(ntile):
                            q, i = t // tq, t % tq
                            nc.tensor.matmul(
                                out=blk_ps,
                                lhsT=G_ts[q][:, b * tq + i, 0:c + 1],
                                rhs=indw_ts[(b, t)][:],
                                start=(t == 0), stop=(t == ntile - 1))
                        bsb = wp.tile([65, P], bf, tag="bsb")
                        nc.scalar.activation(out=bsb[:], in_=blk_ps,
                                             func=AF.Copy)
                        denc_ps = S[:, 384:385]
                        nc.tensor.matmul(out=denc_ps, lhsT=bsb[64:65, :],
                                         rhs=ones_t[64:65, 0:1],
                                         start=True, stop=True)
                        post_ps = S[:, 448:448 + c]
                        nc.tensor.matmul(out=post_ps, lhsT=bsb[0:64, :],
                                         rhs=Wl_t, start=True, stop=True)
                        dpe = wp.tile([P, 1], fp, tag="dpe")
                        nc.vector.tensor_scalar_add(out=dpe[:], in0=denc_ps,
                                                    scalar1=1e-16)
                        rec = wp.tile([P, 1], fp, tag="rec")
                        nc.vector.reciprocal(out=rec[:], in_=dpe[:])
                        sc = wp.tile([P, c], fp, tag="sc")
                        nc.vector.tensor_scalar(out=sc[:], in0=post_ps,
                                                scalar1=rec[:, 0:1],
                                                scalar2=None, op0=ALU.mult)
                        outt = wp.tile([P, c], fp, tag="outt")
                        nc.vector.tensor_tensor(out=outt[:], in0=sc[:],
                                                in1=bias_t[:], op=ALU.add)
                        nc.scalar.dma_start(
                            out=dd["out"][ds((2 * pv + b) * P, P), :],
                            in_=outt[:])

                    do_trans(0)
                    while pending_sc:
                        pending_sc.pop(0)()
                    do_m(0)
                    do_attn(0)
                    do_trans(1)
                    do_scatter(0)
                    do_m(1)
                    do_attn(1)
                    pending_sc.append(lambda: do_scatter(1))

                bodies.append(pair_body)

            def run_iter(pv):
                for body in bodies:
                    body(pv)
                while pending_sc:
                    pending_sc.pop(0)()

            if unroll:
                for pv in range(npairs):
                    run_iter(pv)
            else:
                CHUNK = 12
                for s0 in range(0, npairs, CHUNK):
                    with tc.For_i(s0, min(s0 + CHUNK, npairs), 1,
                                  staggered_reset=True) as pv:
                        run_iter(pv)

    nc.compile()
    return nc, dirs


def kernel(x0, x1, edge_index, edge_attr,
           Wl_b, Wr_b, We_b, att_b, b_b,
           Wl_f, Wr_f, We_f, att_f, b_f):
    x0 = np.asarray(x0, np.float32)
    x1 = np.asarray(x1, np.float32)
    edge_attr = np.asarray(edge_attr, np.float32)
    ei = np.asarray(edge_index)
    src, dst = ei[0].astype(np.int64), ei[1].astype(np.int64)

    N, d = x0.shape
    de = edge_attr.shape[1]
    c = np.asarray(Wl_b).shape[1]
    qsize = _ceil_div(N, NQ)
    assert qsize <= 32767

    cores_b, npc, npc_pad, nblk = _prep_direction(x1, src, dst, edge_attr, NCORES)
    cores_f, _, _, _ = _prep_direction(x0, dst, src, edge_attr, NCORES)

    tq = _ceil_div(max(_max_run(cores_b, qsize, nblk),
                       _max_run(cores_f, qsize, nblk)), P)

    lay_b = _layout_direction(cores_b, nblk, tq, qsize, de)
    lay_f = _layout_direction(cores_f, nblk, tq, qsize, de)

    nc, dirs = _build_program(nblk, tq, npc_pad, qsize, de, d, c)

    def tabs(x):
        t = []
        for q in range(NQ):
            xx = x[q * qsize:(q + 1) * qsize]
            rows = np.zeros((qsize, d), np.float32)
            rows[:xx.shape[0]] = xx
            t.append(rows)
        return t

    x0t, x1t = tabs(x0), tabs(x1)
    cstb = np.zeros((P, 771), np.float32)
    cstb[:, 0:P] = np.arange(P, dtype=np.float32)[None, :]      # iota
    cstb[:, P:2 * P] = np.eye(P, dtype=np.float32)              # ident
    cstb[:, 2 * P:2 * P + 1] = 1.0                              # ones
    for di, (Wl, Wr, We, att) in enumerate(
            ((Wl_b, Wr_b, We_b, att_b), (Wl_f, Wr_f, We_f, att_f))):
        o = 257 + di * 257
        Wl = np.asarray(Wl, np.float32)
        cstb[0:d, o:o + c] = Wl
        cstb[:, o + c:o + 2 * c] = np.tile(Wl, (2, 1))
        cstb[0:d, o + 2 * c:o + 3 * c] = np.asarray(Wr, np.float32)
        cstb[0:de, o + 3 * c:o + 4 * c] = np.asarray(We, np.float32)
        cstb[:, o + 4 * c:o + 4 * c + 1] = np.tile(
            np.asarray(att, np.float32).reshape(c, 1), (2, 1))
    cstb = cstb.astype(bfloat16)
    cstf = np.zeros((P, 129), np.float32)
    cstf[:, 0:1] = np.arange(P, dtype=np.float32)[:, None]      # iotac
    cstf[:, 1:1 + c] = np.asarray(b_b, np.float32)[None, :]
    cstf[:, 1 + c:1 + 2 * c] = np.asarray(b_f, np.float32)[None, :]

    def xdT_shard(xd, k):
        lo = k * npc
        sh = xd[lo:min(lo + npc, N)]
        pad = np.zeros((npc_pad, d), np.float32)
        pad[:sh.shape[0]] = sh
        return np.ascontiguousarray(pad.T).astype(bfloat16)

    in_maps = []
    for k in range(NCORES):
        (idxw_b, doffC_b, eaT_b, doffR_b) = lay_b[k]
        (idxw_f, doffC_f, eaT_f, doffR_f) = lay_f[k]
        m = {
            "cstb": cstb, "cstf": cstf,
            "idxw_b": idxw_b, "doffC_b": doffC_b, "eaT_b": eaT_b,
            "doffR_b": doffR_b,
            "idxw_f": idxw_f, "doffC_f": doffC_f, "eaT_f": eaT_f,
            "doffR_f": doffR_f,
            "xdT_b": xdT_shard(x1, k), "xdT_f": xdT_shard(x0, k),
        }
        for q in range(NQ):
            m[f"tab{q}_b"] = x0t[q]
            m[f"tab{q}_f"] = x1t[q]
        in_maps.append(m)

    kernel.last_tq, kernel.last_nblk = tq, nblk
    kernel.last_npc_pad, kernel.last_qsize = npc_pad, qsize
    res = run_bass_kernel_spmd(nc, in_maps, list(range(NCORES)))

    out_b = np.concatenate([res.results[k]["out_b"][:npc]
                            for k in range(NCORES)])[:N]
    out_f = np.concatenate([res.results[k]["out_f"][:npc]
                            for k in range(NCORES)])[:N]
    return (out_b, out_f)


# revision 26
# speedup vs baseline: 6.3080x; 1.0638x over previous
"""Bidirectional GATv2Conv (heads=1) on 8 Trainium2 NeuronCores.

Strategy (edge-parallel, dst-sharded -- no collectives):
- dst nodes range-sharded across 8 cores; each core owns every edge whose
  aggregation target is in its range, so segment-softmax stats stay local.
- Edges sorted by (block-pair, src-quarter, block); each (pair, quarter,
  block) run padded to tq tiles of 128 edges -> identical SPMD program.
- All PE/DVE data in bf16 (tolerance 2e-2 >> bf16 error):
    * node tables hold 128-wide rows [x | 1 | 0...]; plain dma_gather
      (elem 256B) yields G [128e, 128] whose col 64 is the ones column
      used to fold the softmax denominator into the scatter matmul.
    * gT via paired PE transposes packed 4-per-PSUM-bank, single
      [128,512] activation-copy escape (8 tiles per escape).
    * m computed c-major in packed PSUM banks (2 tiles per 128
      partitions); single Lrelu(alpha=0.2) escape per bank.
    * xr[dst] delivered via slot-major indicator: dstoff replicated
      across partitions by a stride-0 DMA (uint8), one DVE is_equal per
      512 cols; xr itself precomputed per-core into persistent SBUF.
    * logits batched per block into one PSUM tile; one exp per block.
    * scatter: indw = (iota==dstoff)*ex fused DVE op per tile; numerator
      and denominator accumulate via one [65,128] matmul per tile.
- out = (num/den) @ Wl + bias; Wl applied after aggregation
  (sum_e alpha_e * (x@Wl) == (sum_e alpha_e x) @ Wl).
"""

import numpy as np
from ml_dtypes import bfloat16

import concourse.bass as bass
import concourse.bacc as bacc
import concourse.mybir as mybir
import concourse.tile as tile
from concourse.bass import ds, AP
from concourse.bass_utils import run_bass_kernel_spmd

P = 128
NCORES = 8
NQ = 4           # src-table quarters (int16 idx limit: 32767 >= 25000)


def _ceil_div(a, b):
    return (a + b - 1) // b


def _prep_direction(x_dst, src, dst, ea, n_cores):
    """Per-core edge bucketing (before padding, which needs global TQ)."""
    N = x_dst.shape[0]
    npc = _ceil_div(N, n_cores)
    npc_pad = _ceil_div(npc, P) * P
    nblk = npc_pad // P
    cores = []
    for k in range(n_cores):
        lo = k * npc
        hi = min(lo + npc, N)
        sel = (dst >= lo) & (dst < hi)
        cores.append((src[sel], dst[sel] - lo, ea[sel]))
    return cores, npc, npc_pad, nblk


def _max_run(cores, qsize, nblk):
    m = 0
    for (e_src, e_dst, e_ea) in cores:
        blk = e_dst >> 7
        qua = e_src // qsize
        key = blk * NQ + qua
        cnt = np.bincount(key, minlength=nblk * NQ)
        m = max(m, int(cnt.max()))
    return m


def _layout_direction(cores, nblk, tq, qsize, de):
    """Build padded per-core device arrays (slot order: pair,quarter,block)."""
    npairs = nblk // 2
    run = tq * P
    slots_pair = NQ * 2 * run
    total = npairs * slots_pair
    nrun = npairs * NQ * 2
    out = []
    for (e_src, e_dst, e_ea) in cores:
        blk = e_dst >> 7
        qua = e_src // qsize
        runid = ((blk >> 1) * NQ + qua) * 2 + (blk & 1)
        order = np.argsort(runid, kind="stable")
        s_src = e_src[order]
        s_dst = e_dst[order]
        s_ea = e_ea[order]
        s_run = runid[order]
        s_loc = (s_src - (s_src // qsize) * qsize).astype(np.int16)
        s_off = (s_dst & 127).astype(np.uint8)

        idx_all = np.zeros(total, np.int16)
        doff_u8 = np.full(total, 255, np.uint8)
        doff_f = np.full(total, -1.0, np.float32)
        ea_all = np.zeros((total, de), np.float32)
        starts = np.searchsorted(s_run, np.arange(nrun + 1))
        for r in range(nrun):
            s0, s1 = int(starts[r]), int(starts[r + 1])
            cnt = s1 - s0
            assert cnt <= run, f"run {r} has {cnt} > {run} edges"
            base = r * run
            idx_all[base:base + cnt] = s_loc[s0:s1]
            doff_u8[base:base + cnt] = s_off[s0:s1]
            doff_f[base:base + cnt] = s_off[s0:s1]
            ea_all[base:base + cnt] = s_ea[s0:s1]

        idxw = np.tile(idx_all.reshape(-1, 16).T, (8, 1)).copy()
        doffC = doff_f.reshape(-1, P).T.copy()               # [128, total/128]
        eaT = np.ascontiguousarray(ea_all.T).astype(bfloat16)  # [de, total]
        doffR = doff_u8.reshape(1, total)
        out.append((idxw, doffC, eaT, doffR))
    return out


def _build_program(nblk, tq, npc_pad, qsize, de, d, c, unroll=False):
    assert nblk % 2 == 0
    npairs = nblk // 2
    ntile = NQ * tq
    slots_pair = NQ * 2 * tq * P
    fp = mybir.dt.float32
    bf = mybir.dt.bfloat16
    u8 = mybir.dt.uint8
    i16 = mybir.dt.int16
    AF = mybir.ActivationFunctionType
    ALU = mybir.AluOpType
    nc = bacc.Bacc("TRN2")

    def dram(name, shape, dt=fp, out=False):
        return nc.declare_dram_parameter(name, list(shape), dt, isOutput=out)

    dirs = {}
    for dn in ("b", "f"):
        dirs[dn] = dict(
            q_tabs=[dram(f"tab{q}_{dn}", [qsize, d]) for q in range(NQ)],
            idxw=dram(f"idxw_{dn}", [P, npairs * slots_pair // 16], i16),
            doffC=dram(f"doffC_{dn}", [P, npairs * 2 * ntile]),
            doffR=dram(f"doffR_{dn}", [1, npairs * slots_pair], u8),
            eaT=dram(f"eaT_{dn}", [de, npairs * slots_pair], bf),
            xdT=dram(f"xdT_{dn}", [d, npc_pad], bf),
            out=dram(f"out_{dn}", [npc_pad, c], out=True),
        )
    cstb_d = dram("cstb", [P, 771], bf)   # packed bf16 consts
    cstf_d = dram("cstf", [P, 129])       # packed fp32 consts

    # transpose slot list per block: (quarter, first tile, tiles in pair)
    tlist = []
    for q in range(NQ):
        for i0 in range(0, tq, 2):
            tlist.append((q, i0, min(2, tq - i0)))
    ntrans = len(tlist)
    tpq = _ceil_div(tq, 2)                      # transposes per quarter
    nbank_t = _ceil_div(ntrans, 4)              # gT psum banks per block
    nbank_m = _ceil_div(ntile, 8)               # m psum banks per block

    def t_slot(q, i):
        """tile (q, i) -> (gT bank, col, partition base)"""
        j = q * tpq + i // 2
        return j // 4, (j % 4) * P, (i % 2) * 64

    def m_slot(t):
        """tile t -> (m bank, partition base, col)"""
        return t // 8, (t % 2) * 64, ((t // 2) % 4) * P

    with tile.TileContext(nc) as tc:
        with tc.tile_pool(name="const", bufs=1) as cp, \
             tc.tile_pool(name="xr", bufs=1) as xp, \
             tc.tile_pool(name="load", bufs=2) as lp, \
             tc.tile_pool(name="gt", bufs=3) as gp, \
             tc.tile_pool(name="m8", bufs=3) as mp8, \
             tc.tile_pool(name="ind", bufs=20) as ip, \
             tc.tile_pool(name="work", bufs=4) as wp, \
             tc.tile_pool(name="indw", bufs=24) as iw, \
             tc.tile_pool(name="ps_t", bufs=3, space="PSUM") as ps_t, \
             tc.tile_pool(name="ps_m", bufs=3, space="PSUM") as ps_m, \
             tc.tile_pool(name="ps_s", bufs=2, space="PSUM") as ps_s:

            cstb_t = cp.tile([P, 771], bf)
            nc.sync.dma_start(out=cstb_t[:], in_=cstb_d[:])
            cstf_t = cp.tile([P, 129], fp)
            nc.sync.dma_start(out=cstf_t[:], in_=cstf_d[:])
            iota_t = cstb_t[:, 0:P]
            ident_t = cstb_t[:, P:2 * P]
            ones_t = cstb_t[:, 2 * P:2 * P + 1]
            iotac_t = cstf_t[:, 0:1]

            bodies = []
            pending_sc = []
            for di, dn in enumerate(("b", "f")):
                dd = dirs[dn]
                o = 257 + di * 257
                Wl_t = cstb_t[0:d, o:o + c]
                Wl2_t = cstb_t[:, o + c:o + 2 * c]
                Wr_t = cstb_t[0:d, o + 2 * c:o + 3 * c]
                We_t = cstb_t[0:de, o + 3 * c:o + 4 * c]
                att2_t = cstb_t[:, o + 4 * c:o + 4 * c + 1]
                bias_t = cstf_t[:, 1 + di * c:1 + (di + 1) * c]

                # ---- prologue: xr = x_dst @ Wr for all own blocks ----
                xr_sb = xp.tile([P, nblk * c], bf, tag=f"xr{dn}")
                for g in range(_ceil_div(nblk, 8)):
                    nb = min(8, nblk - 8 * g)
                    xd_t = lp.tile([d, 8 * P], bf, tag="xd")
                    nc.scalar.dma_start(out=xd_t[:, 0:nb * P],
                                        in_=dd["xdT"][:, ds(g * 8 * P, nb * P)])
                    pro_ps = ps_m.tile([P, 512], fp, tag="mb")
                    for j in range(nb):
                        nc.tensor.matmul(out=pro_ps[:, j * c:(j + 1) * c],
                                         lhsT=xd_t[:, j * P:(j + 1) * P],
                                         rhs=Wr_t, start=True, stop=True)
                    nc.scalar.activation(out=xr_sb[:, ds(g * 8 * c, nb * c)],
                                         in_=pro_ps[:, 0:nb * c], func=AF.Copy)

                def pair_body(pv, dd=dd, Wl_t=Wl_t, Wl2_t=Wl2_t,
                              We_t=We_t, att2_t=att2_t, bias_t=bias_t,
                              xr_sb=xr_sb):
                    base = pv * slots_pair
                    idxw_t = lp.tile([P, slots_pair // 16], i16, tag="idxw")
                    nc.sync.dma_start(
                        out=idxw_t[:],
                        in_=dd["idxw"][:, ds(pv * (slots_pair // 16),
                                             slots_pair // 16)])
                    ea_t = lp.tile([de, slots_pair], bf, tag="ea")
                    nc.sync.dma_start(out=ea_t[:],
                                      in_=dd["eaT"][:, ds(base, slots_pair)])
                    dC_t = lp.tile([P, 2 * ntile], fp, tag="dC")
                    nc.scalar.dma_start(out=dC_t[:],
                                      in_=dd["doffC"][:, ds(pv * 2 * ntile,
                                                            2 * ntile)])
                    dR_t = lp.tile([P, slots_pair], u8, tag="dR")
                    src = dd["doffR"][0:1, ds(base, slots_pair)]
                    rep = AP(src.tensor, src.offset, [[0, P]] + src.ap[1:])
                    nc.sync.dma_start(out=dR_t[:], in_=rep)

                    xr_pair = lp.tile([P, 2 * c], bf, tag="xrp")
                    nc.scalar.dma_start(out=xr_pair[:],
                                      in_=xr_sb[:, ds(pv * 2 * c, 2 * c)])
                    G_ts = []
                    for q in range(NQ):
                        G32 = lp.tile([P, 2 * tq, d], fp, tag=f"G32_{q}")
                        for b in (0, 1):
                            run = q * 2 + b
                            nc.gpsimd.dma_gather(
                                out_ap=G32[:, b * tq:(b + 1) * tq, :],
                                in_ap=dd["q_tabs"][q][:],
                                idxs_ap=idxw_t[:, run * (tq * 8):
                                               (run + 1) * (tq * 8)],
                                num_idxs=tq * P,
                                num_idxs_reg=tq * P,
                                elem_size=d,
                            )
                        G = lp.tile([P, 2 * tq, c + 1], bf, tag=f"G{q}")
                        nc.vector.memset(G[:, :, c:c + 1], 1.0)
                        nc.vector.tensor_scalar_mul(out=G[:, :, 0:c],
                                                    in0=G32[:, :, :],
                                                    scalar1=1.0)
                        G_ts.append(G)

                    # slot-major indicator, one tile per (quarter, block)
                    # run, block-0 runs emitted first so m(0) is fed early
                    run_sz = tq * P
                    indT_ch = [None] * (2 * NQ)
                    for run in [q * 2 + b for b in (0, 1) for q in range(NQ)]:
                        it = ip.tile([P, run_sz], bf, tag="indT", name="indT")
                        nc.vector.tensor_scalar(
                            out=it[:],
                            in0=dR_t[:, ds(run * run_sz, run_sz)],
                            scalar1=iotac_t, scalar2=None,
                            op0=ALU.is_equal)
                        indT_ch[run] = it

                    # skewed per-block phases: block 1's transposes are
                    # emitted between block 0's attention and scatter so PE
                    # stays busy while exp/indw cook on ACT/DVE.
                    gT8s = {}
                    m8s = {}
                    ex_ts = {}
                    S_ts = {}
                    indw_ts = {}

                    def do_trans(b):
                        for k in range(nbank_t):
                            tbank = ps_t.tile([P, 512], bf, tag="tb", name="tb")
                            for j, (q, i0, w) in enumerate(tlist):
                                if j // 4 != k:
                                    continue
                                for u in range(w):
                                    nc.tensor.transpose(
                                        out=tbank[u * 64:u * 64 + 64,
                                                  ds((j % 4) * P, P)],
                                        in_=G_ts[q][:, b * tq + i0 + u, 0:64],
                                        identity=ident_t)
                            g8 = gp.tile([P, 512], bf, tag="g8")
                            if k % 2 == 0:
                                nc.scalar.activation(out=g8[:], in_=tbank[:],
                                                     func=AF.Copy)
                            else:
                                nc.vector.tensor_scalar_mul(out=g8[:],
                                                            in0=tbank[:],
                                                            scalar1=1.0)
                            gT8s[(b, k)] = g8

                    def do_m(b):
                        mbanks = [ps_m.tile([P, 512], fp, tag="mb", name="mb")
                                  for _ in range(nbank_m)]
                        for t in range(ntile):
                            q, i = t // tq, t % tq
                            tb, tcol, tpb = t_slot(q, i)
                            mb, mpb, mcol = m_slot(t)
                            dst = mbanks[mb][mpb:mpb + c, ds(mcol, P)]
                            rhs_g = gT8s[(b, tb)][tpb:tpb + 64, ds(tcol, P)]
                            scol = ((q * 2 + b) * tq + i) * P
                            nc.tensor.matmul(out=dst,
                                             lhsT=Wl2_t[tpb:tpb + d, :],
                                             rhs=rhs_g,
                                             start=True, stop=False)
                            nc.tensor.matmul(out=dst, lhsT=We_t,
                                             rhs=ea_t[:, ds(scol, P)],
                                             start=False, stop=False)
                            nc.tensor.matmul(out=dst,
                                             lhsT=xr_pair[:, b * c:(b + 1) * c],
                                             rhs=indT_ch[q * 2 + b]
                                             [:, ds(i * P, P)],
                                             start=False, stop=True)
                            if (t + 1) % 8 == 0 or t == ntile - 1:
                                k = t // 8
                                n_t = min(8, ntile - 8 * k)
                                cols = _ceil_div(n_t, 2) * P
                                m8 = mp8.tile([P, 512], bf, tag="m8")
                                nc.scalar.activation(out=m8[:, 0:cols],
                                                     in_=mbanks[k][:, 0:cols],
                                                     func=AF.Prelu, alpha=0.2)
                                m8s[(b, k)] = m8

                    def do_attn(b):
                        S = ps_s.tile([P, 512], fp, tag="sb", name="sb")
                        S_ts[b] = S
                        lg_ps = S[:, 0:ntile]
                        for t in range(ntile):
                            mb, mpb, mcol = m_slot(t)
                            nc.tensor.matmul(
                                out=lg_ps[:, t:t + 1],
                                lhsT=m8s[(b, mb)][mpb:mpb + c, ds(mcol, P)],
                                rhs=att2_t[mpb:mpb + c, :],
                                start=True, stop=True)
                        ex_t = wp.tile([P, ntile], fp, tag="ex")
                        nc.scalar.activation(out=ex_t[:], in_=lg_ps[:],
                                             func=AF.Exp)
                        ex_ts[b] = ex_t
                        for t in range(ntile):
                            q, i = t // tq, t % tq
                            indw = iw.tile([P, P], bf, tag="iw")
                            nc.vector.tensor_scalar(
                                out=indw[:], in0=iota_t,
                                scalar1=dC_t[:, (q * 2 + b) * tq + i:
                                             (q * 2 + b) * tq + i + 1],
                                scalar2=ex_t[:, t:t + 1],
                                op0=ALU.is_equal, op1=ALU.mult)
                            indw_ts[(b, t)] = indw

                    def do_scatter(b):
                        S = S_ts[b]
                        blk_ps = S[0:65, 256:384]
                        for t in range(ntile):
                            q, i = t // tq, t % tq
                            nc.tensor.matmul(
                                out=blk_ps,
                                lhsT=G_ts[q][:, b * tq + i, 0:c + 1],
                                rhs=indw_ts[(b, t)][:],
                                start=(t == 0), stop=(t == ntile - 1))
                        bsb = wp.tile([65, P], bf, tag="bsb")
                        nc.scalar.activation(out=bsb[:], in_=blk_ps,
                                             func=AF.Copy)
                        denc_ps = S[:, 384:385]
                        nc.tensor.matmul(out=denc_ps, lhsT=bsb[64:65, :],
                                         rhs=ones_t[64:65, 0:1],
                                         start=True, stop=True)
                        post_ps = S[:, 448:448 + c]
                        nc.tensor.matmul(out=post_ps, lhsT=bsb[0:64, :],
                                         rhs=Wl_t, start=True, stop=True)
                        dpe = wp.tile([P, 1], fp, tag="dpe")
                        nc.vector.tensor_scalar_add(out=dpe[:], in0=denc_ps,
                                                    scalar1=1e-16)
                        rec = wp.tile([P, 1], fp, tag="rec")
                        nc.vector.reciprocal(out=rec[:], in_=dpe[:])
                        sc = wp.tile([P, c], fp, tag="sc")
                        nc.vector.tensor_scalar(out=sc[:], in0=post_ps,
                                                scalar1=rec[:, 0:1],
                                                scalar2=None, op0=ALU.mult)
                        outt = wp.tile([P, c], fp, tag="outt")
                        nc.vector.tensor_tensor(out=outt[:], in0=sc[:],
                                                in1=bias_t[:], op=ALU.add)
                        nc.sync.dma_start(
                            out=dd["out"][ds((2 * pv + b) * P, P), :],
                            in_=outt[:])

                    do_trans(0)
                    while pending_sc:
                        pending_sc.pop(0)()
                    do_m(0)
                    do_attn(0)
                    do_trans(1)
                    do_scatter(0)
                    do_m(1)
                    do_attn(1)
                    pending_sc.append(lambda: do_scatter(1))

                bodies.append(pair_body)

            def run_iter(pv):
                for body in bodies:
                    body(pv)
                while pending_sc:
                    pending_sc.pop(0)()

            if unroll:
                for pv in range(npairs):
                    run_iter(pv)
            else:
                CHUNK = 12
                for s0 in range(0, npairs, CHUNK):
                    with tc.For_i(s0, min(s0 + CHUNK, npairs), 1,
                                  staggered_reset=True) as pv:
                        run_iter(pv)

    nc.compile()
    return nc, dirs


def kernel(x0, x1, edge_index, edge_attr,
           Wl_b, Wr_b, We_b, att_b, b_b,
           Wl_f, Wr_f, We_f, att_f, b_f):
    x0 = np.asarray(x0, np.float32)
    x1 = np.asarray(x1, np.float32)
    edge_attr = np.asarray(edge_attr, np.float32)
    ei = np.asarray(edge_index)
    src, dst = ei[0].astype(np.int64), ei[1].astype(np.int64)

    N, d = x0.shape
    de = edge_attr.shape[1]
    c = np.asarray(Wl_b).shape[1]
    qsize = _ceil_div(N, NQ)
    assert qsize <= 32767

    cores_b, npc, npc_pad, nblk = _prep_direction(x1, src, dst, edge_attr, NCORES)
    cores_f, _, _, _ = _prep_direction(x0, dst, src, edge_attr, NCORES)

    tq = _ceil_div(max(_max_run(cores_b, qsize, nblk),
                       _max_run(cores_f, qsize, nblk)), P)

    lay_b = _layout_direction(cores_b, nblk, tq, qsize, de)
    lay_f = _layout_direction(cores_f, nblk, tq, qsize, de)

    nc, dirs = _build_program(nblk, tq, npc_pad, qsize, de, d, c)

    def tabs(x):
        t = []
        for q in range(NQ):
            xx = x[q * qsize:(q + 1) * qsize]
            rows = np.zeros((qsize, d), np.float32)
            rows[:xx.shape[0]] = xx
            t.append(rows)
        return t

    x0t, x1t = tabs(x0), tabs(x1)
    cstb = np.zeros((P, 771), np.float32)
    cstb[:, 0:P] = np.arange(P, dtype=np.float32)[None, :]      # iota
    cstb[:, P:2 * P] = np.eye(P, dtype=np.float32)              # ident
    cstb[:, 2 * P:2 * P + 1] = 1.0                              # ones
    for di, (Wl, Wr, We, att) in enumerate(
            ((Wl_b, Wr_b, We_b, att_b), (Wl_f, Wr_f, We_f, att_f))):
        o = 257 + di * 257
        Wl = np.asarray(Wl, np.float32)
        cstb[0:d, o:o + c] = Wl
        cstb[:, o + c:o + 2 * c] = np.tile(Wl, (2, 1))
        cstb[0:d, o + 2 * c:o + 3 * c] = np.asarray(Wr, np.float32)
        cstb[0:de, o + 3 * c:o + 4 * c] = np.asarray(We, np.float32)
        cstb[:, o + 4 * c:o + 4 * c + 1] = np.tile(
            np.asarray(att, np.float32).reshape(c, 1), (2, 1))
    cstb = cstb.astype(bfloat16)
    cstf = np.zeros((P, 129), np.float32)
    cstf[:, 0:1] = np.arange(P, dtype=np.float32)[:, None]      # iotac
    cstf[:, 1:1 + c] = np.asarray(b_b, np.float32)[None, :]
    cstf[:, 1 + c:1 + 2 * c] = np.asarray(b_f, np.float32)[None, :]

    def xdT_shard(xd, k):
        lo = k * npc
        sh = xd[lo:min(lo + npc, N)]
        pad = np.zeros((npc_pad, d), np.float32)
        pad[:sh.shape[0]] = sh
        return np.ascontiguousarray(pad.T).astype(bfloat16)

    in_maps = []
    for k in range(NCORES):
        (idxw_b, doffC_b, eaT_b, doffR_b) = lay_b[k]
        (idxw_f, doffC_f, eaT_f, doffR_f) = lay_f[k]
        m = {
            "cstb": cstb, "cstf": cstf,
            "idxw_b": idxw_b, "doffC_b": doffC_b, "eaT_b": eaT_b,
            "doffR_b": doffR_b,
            "idxw_f": idxw_f, "doffC_f": doffC_f, "eaT_f": eaT_f,
            "doffR_f": doffR_f,
            "xdT_b": xdT_shard(x1, k), "xdT_f": xdT_shard(x0, k),
        }
        for q in range(NQ):
            m[f"tab{q}_b"] = x0t[q]
            m[f"tab{q}_f"] = x1t[q]
        in_maps.append(m)

    kernel.last_tq, kernel.last_nblk = tq, nblk
    kernel.last_npc_pad, kernel.last_qsize = npc_pad, qsize
    res = run_bass_kernel_spmd(nc, in_maps, list(range(NCORES)))

    out_b = np.concatenate([res.results[k]["out_b"][:npc]
                            for k in range(NCORES)])[:N]
    out_f = np.concatenate([res.results[k]["out_f"][:npc]
                            for k in range(NCORES)])[:N]
    return (out_b, out_f)


# revision 27
# speedup vs baseline: 6.4517x; 1.0228x over previous
"""Bidirectional GATv2Conv (heads=1) on 8 Trainium2 NeuronCores.

Strategy (edge-parallel, dst-sharded -- no collectives):
- dst nodes range-sharded across 8 cores; each core owns every edge whose
  aggregation target is in its range, so segment-softmax stats stay local.
- Edges sorted by (block-pair, src-quarter, block); each (pair, quarter,
  block) run padded to tq tiles of 128 edges -> identical SPMD program.
- All PE/DVE data in bf16 (tolerance 2e-2 >> bf16 error):
    * node tables hold 128-wide rows [x | 1 | 0...]; plain dma_gather
      (elem 256B) yields G [128e, 128] whose col 64 is the ones column
      used to fold the softmax denominator into the scatter matmul.
    * gT via paired PE transposes packed 4-per-PSUM-bank, single
      [128,512] activation-copy escape (8 tiles per escape).
    * m computed c-major in packed PSUM banks (2 tiles per 128
      partitions); single Lrelu(alpha=0.2) escape per bank.
    * xr[dst] delivered via slot-major indicator: dstoff replicated
      across partitions by a stride-0 DMA (uint8), one DVE is_equal per
      512 cols; xr itself precomputed per-core into persistent SBUF.
    * logits batched per block into one PSUM tile; one exp per block.
    * scatter: indw = (iota==dstoff)*ex fused DVE op per tile; numerator
      and denominator accumulate via one [65,128] matmul per tile.
- out = (num/den) @ Wl + bias; Wl applied after aggregation
  (sum_e alpha_e * (x@Wl) == (sum_e alpha_e x) @ Wl).
"""

import numpy as np
from ml_dtypes import bfloat16

import concourse.bass as bass
import concourse.bacc as bacc
import concourse.mybir as mybir
import concourse.tile as tile
from concourse.bass import ds, AP
from concourse.bass_utils import run_bass_kernel_spmd

P = 128
NCORES = 8
NQ = 4           # src-table quarters (int16 idx limit: 32767 >= 25000)


def _ceil_div(a, b):
    return (a + b - 1) // b


def _prep_direction(x_dst, src, dst, ea, n_cores):
    """Per-core edge bucketing (before padding, which needs global TQ)."""
    N = x_dst.shape[0]
    npc = _ceil_div(N, n_cores)
    npc_pad = _ceil_div(npc, P) * P
    nblk = npc_pad // P
    cores = []
    for k in range(n_cores):
        lo = k * npc
        hi = min(lo + npc, N)
        sel = (dst >= lo) & (dst < hi)
        cores.append((src[sel], dst[sel] - lo, ea[sel]))
    return cores, npc, npc_pad, nblk


def _max_run(cores, qsize, nblk):
    m = 0
    for (e_src, e_dst, e_ea) in cores:
        blk = e_dst >> 7
        qua = e_src // qsize
        key = blk * NQ + qua
        cnt = np.bincount(key, minlength=nblk * NQ)
        m = max(m, int(cnt.max()))
    return m


def _layout_direction(cores, nblk, tq, qsize, de):
    """Build padded per-core device arrays (slot order: pair,quarter,block)."""
    npairs = nblk // 2
    run = tq * P
    slots_pair = NQ * 2 * run
    total = npairs * slots_pair
    nrun = npairs * NQ * 2
    out = []
    for (e_src, e_dst, e_ea) in cores:
        blk = e_dst >> 7
        qua = e_src // qsize
        runid = ((blk >> 1) * NQ + qua) * 2 + (blk & 1)
        order = np.argsort(runid, kind="stable")
        s_src = e_src[order]
        s_dst = e_dst[order]
        s_ea = e_ea[order]
        s_run = runid[order]
        s_loc = (s_src - (s_src // qsize) * qsize).astype(np.int16)
        s_off = (s_dst & 127).astype(np.uint8)

        idx_all = np.zeros(total, np.int16)
        doff_u8 = np.full(total, 255, np.uint8)
        doff_f = np.full(total, -1.0, np.float32)
        ea_all = np.zeros((total, de), np.float32)
        starts = np.searchsorted(s_run, np.arange(nrun + 1))
        for r in range(nrun):
            s0, s1 = int(starts[r]), int(starts[r + 1])
            cnt = s1 - s0
            assert cnt <= run, f"run {r} has {cnt} > {run} edges"
            base = r * run
            idx_all[base:base + cnt] = s_loc[s0:s1]
            doff_u8[base:base + cnt] = s_off[s0:s1]
            doff_f[base:base + cnt] = s_off[s0:s1]
            ea_all[base:base + cnt] = s_ea[s0:s1]

        idxw = np.tile(idx_all.reshape(-1, 16).T, (8, 1)).copy()
        doffC = doff_f.reshape(-1, P).T.copy()               # [128, total/128]
        eaT = np.ascontiguousarray(ea_all.T).astype(bfloat16)  # [de, total]
        doffR = doff_u8.reshape(1, total)
        out.append((idxw, doffC, eaT, doffR))
    return out


def _build_program(nblk, tq, npc_pad, qsize, de, d, c, unroll=False):
    assert nblk % 2 == 0
    npairs = nblk // 2
    ntile = NQ * tq
    slots_pair = NQ * 2 * tq * P
    fp = mybir.dt.float32
    bf = mybir.dt.bfloat16
    u8 = mybir.dt.uint8
    i16 = mybir.dt.int16
    AF = mybir.ActivationFunctionType
    ALU = mybir.AluOpType
    nc = bacc.Bacc("TRN2")

    def dram(name, shape, dt=fp, out=False):
        return nc.declare_dram_parameter(name, list(shape), dt, isOutput=out)

    dirs = {}
    for dn in ("b", "f"):
        dirs[dn] = dict(
            q_tabs=[dram(f"tab{q}_{dn}", [qsize, d]) for q in range(NQ)],
            idxw=dram(f"idxw_{dn}", [P, npairs * slots_pair // 16], i16),
            doffC=dram(f"doffC_{dn}", [P, npairs * 2 * ntile]),
            doffR=dram(f"doffR_{dn}", [1, npairs * slots_pair], u8),
            eaT=dram(f"eaT_{dn}", [de, npairs * slots_pair], bf),
            xdT=dram(f"xdT_{dn}", [d, npc_pad], bf),
            out=dram(f"out_{dn}", [npc_pad, c], out=True),
        )
    cstb_d = dram("cstb", [P, 771], bf)   # packed bf16 consts
    cstf_d = dram("cstf", [P, 129])       # packed fp32 consts

    # transpose slot list per block: (quarter, first tile, tiles in pair)
    tlist = []
    for q in range(NQ):
        for i0 in range(0, tq, 2):
            tlist.append((q, i0, min(2, tq - i0)))
    ntrans = len(tlist)
    tpq = _ceil_div(tq, 2)                      # transposes per quarter
    nbank_t = _ceil_div(ntrans, 4)              # gT psum banks per block
    nbank_m = _ceil_div(ntile, 8)               # m psum banks per block

    def t_slot(q, i):
        """tile (q, i) -> (gT bank, col, partition base)"""
        j = q * tpq + i // 2
        return j // 4, (j % 4) * P, (i % 2) * 64

    def m_slot(t):
        """tile t -> (m bank, partition base, col)"""
        return t // 8, (t % 2) * 64, ((t // 2) % 4) * P

    with tile.TileContext(nc) as tc:
        with tc.tile_pool(name="const", bufs=1) as cp, \
             tc.tile_pool(name="xr", bufs=1) as xp, \
             tc.tile_pool(name="load", bufs=2) as lp, \
             tc.tile_pool(name="gt", bufs=3) as gp, \
             tc.tile_pool(name="m8", bufs=3) as mp8, \
             tc.tile_pool(name="ind", bufs=20) as ip, \
             tc.tile_pool(name="work", bufs=4) as wp, \
             tc.tile_pool(name="indw", bufs=24) as iw, \
             tc.tile_pool(name="ps_t", bufs=4, space="PSUM") as ps_t, \
             tc.tile_pool(name="ps_m", bufs=2, space="PSUM") as ps_m, \
             tc.tile_pool(name="ps_s", bufs=2, space="PSUM") as ps_s:

            cstb_t = cp.tile([P, 771], bf)
            nc.sync.dma_start(out=cstb_t[:], in_=cstb_d[:])
            cstf_t = cp.tile([P, 129], fp)
            nc.sync.dma_start(out=cstf_t[:], in_=cstf_d[:])
            iota_t = cstb_t[:, 0:P]
            ident_t = cstb_t[:, P:2 * P]
            ones_t = cstb_t[:, 2 * P:2 * P + 1]
            iotac_t = cstf_t[:, 0:1]

            bodies = []
            pending_sc = []
            for di, dn in enumerate(("b", "f")):
                dd = dirs[dn]
                o = 257 + di * 257
                Wl_t = cstb_t[0:d, o:o + c]
                Wl2_t = cstb_t[:, o + c:o + 2 * c]
                Wr_t = cstb_t[0:d, o + 2 * c:o + 3 * c]
                We_t = cstb_t[0:de, o + 3 * c:o + 4 * c]
                att2_t = cstb_t[:, o + 4 * c:o + 4 * c + 1]
                bias_t = cstf_t[:, 1 + di * c:1 + (di + 1) * c]

                # ---- prologue: xr = x_dst @ Wr for all own blocks ----
                xr_sb = xp.tile([P, nblk * c], bf, tag=f"xr{dn}")
                for g in range(_ceil_div(nblk, 8)):
                    nb = min(8, nblk - 8 * g)
                    xd_t = lp.tile([d, 8 * P], bf, tag="xd")
                    nc.scalar.dma_start(out=xd_t[:, 0:nb * P],
                                        in_=dd["xdT"][:, ds(g * 8 * P, nb * P)])
                    pro_ps = ps_m.tile([P, 512], fp, tag="mb")
                    for j in range(nb):
                        nc.tensor.matmul(out=pro_ps[:, j * c:(j + 1) * c],
                                         lhsT=xd_t[:, j * P:(j + 1) * P],
                                         rhs=Wr_t, start=True, stop=True)
                    nc.scalar.activation(out=xr_sb[:, ds(g * 8 * c, nb * c)],
                                         in_=pro_ps[:, 0:nb * c], func=AF.Copy)

                def pair_body(pv, dd=dd, Wl_t=Wl_t, Wl2_t=Wl2_t,
                              We_t=We_t, att2_t=att2_t, bias_t=bias_t,
                              xr_sb=xr_sb):
                    base = pv * slots_pair
                    idxw_t = lp.tile([P, slots_pair // 16], i16, tag="idxw")
                    nc.sync.dma_start(
                        out=idxw_t[:],
                        in_=dd["idxw"][:, ds(pv * (slots_pair // 16),
                                             slots_pair // 16)])
                    G_ts = []
                    for q in range(NQ):
                        G32 = lp.tile([P, 2 * tq, d], fp, tag=f"G32_{q}")
                        for b in (0, 1):
                            run = q * 2 + b
                            nc.gpsimd.dma_gather(
                                out_ap=G32[:, b * tq:(b + 1) * tq, :],
                                in_ap=dd["q_tabs"][q][:],
                                idxs_ap=idxw_t[:, run * (tq * 8):
                                               (run + 1) * (tq * 8)],
                                num_idxs=tq * P,
                                num_idxs_reg=tq * P,
                                elem_size=d,
                            )
                        G = lp.tile([P, 2 * tq, c + 1], bf, tag=f"G{q}")
                        nc.vector.memset(G[:, :, c:c + 1], 1.0)
                        nc.vector.tensor_scalar_mul(out=G[:, :, 0:c],
                                                    in0=G32[:, :, :],
                                                    scalar1=1.0)
                        G_ts.append(G)

                    ea_t = lp.tile([de, slots_pair], bf, tag="ea")
                    nc.sync.dma_start(out=ea_t[:],
                                      in_=dd["eaT"][:, ds(base, slots_pair)])
                    dC_t = lp.tile([P, 2 * ntile], fp, tag="dC")
                    nc.scalar.dma_start(out=dC_t[:],
                                      in_=dd["doffC"][:, ds(pv * 2 * ntile,
                                                            2 * ntile)])
                    dR_t = lp.tile([P, slots_pair], u8, tag="dR")
                    src = dd["doffR"][0:1, ds(base, slots_pair)]
                    rep = AP(src.tensor, src.offset, [[0, P]] + src.ap[1:])
                    nc.sync.dma_start(out=dR_t[:], in_=rep)

                    xr_pair = lp.tile([P, 2 * c], bf, tag="xrp")
                    nc.scalar.dma_start(out=xr_pair[:],
                                      in_=xr_sb[:, ds(pv * 2 * c, 2 * c)])

                    # slot-major indicator, one tile per (quarter, block)
                    # run, block-0 runs emitted first so m(0) is fed early
                    run_sz = tq * P
                    indT_ch = [None] * (2 * NQ)
                    for run in [q * 2 + b for b in (0, 1) for q in range(NQ)]:
                        it = ip.tile([P, run_sz], bf, tag="indT", name="indT")
                        nc.vector.tensor_scalar(
                            out=it[:],
                            in0=dR_t[:, ds(run * run_sz, run_sz)],
                            scalar1=iotac_t, scalar2=None,
                            op0=ALU.is_equal)
                        indT_ch[run] = it

                    # skewed per-block phases: block 1's transposes are
                    # emitted between block 0's attention and scatter so PE
                    # stays busy while exp/indw cook on ACT/DVE.
                    gT8s = {}
                    m8s = {}
                    ex_ts = {}
                    S_ts = {}
                    indw_ts = {}

                    def do_trans(b):
                        for k in range(nbank_t):
                            tbank = ps_t.tile([P, 512], bf, tag="tb", name="tb")
                            for j, (q, i0, w) in enumerate(tlist):
                                if j // 4 != k:
                                    continue
                                for u in range(w):
                                    nc.tensor.transpose(
                                        out=tbank[u * 64:u * 64 + 64,
                                                  ds((j % 4) * P, P)],
                                        in_=G_ts[q][:, b * tq + i0 + u, 0:64],
                                        identity=ident_t)
                            g8 = gp.tile([P, 512], bf, tag="g8")
                            if k % 2 == 0:
                                nc.scalar.activation(out=g8[:], in_=tbank[:],
                                                     func=AF.Copy)
                            else:
                                nc.vector.tensor_scalar_mul(out=g8[:],
                                                            in0=tbank[:],
                                                            scalar1=1.0)
                            gT8s[(b, k)] = g8

                    def do_m(b):
                        mbanks = [ps_m.tile([P, 512], fp, tag="mb", name="mb")
                                  for _ in range(nbank_m)]
                        for t in range(ntile):
                            q, i = t // tq, t % tq
                            tb, tcol, tpb = t_slot(q, i)
                            mb, mpb, mcol = m_slot(t)
                            dst = mbanks[mb][mpb:mpb + c, ds(mcol, P)]
                            rhs_g = gT8s[(b, tb)][tpb:tpb + 64, ds(tcol, P)]
                            scol = ((q * 2 + b) * tq + i) * P
                            nc.tensor.matmul(out=dst,
                                             lhsT=Wl2_t[tpb:tpb + d, :],
                                             rhs=rhs_g,
                                             start=True, stop=False)
                            nc.tensor.matmul(out=dst, lhsT=We_t,
                                             rhs=ea_t[:, ds(scol, P)],
                                             start=False, stop=False)
                            nc.tensor.matmul(out=dst,
                                             lhsT=xr_pair[:, b * c:(b + 1) * c],
                                             rhs=indT_ch[q * 2 + b]
                                             [:, ds(i * P, P)],
                                             start=False, stop=True)
                            if (t + 1) % 8 == 0 or t == ntile - 1:
                                k = t // 8
                                n_t = min(8, ntile - 8 * k)
                                cols = _ceil_div(n_t, 2) * P
                                m8 = mp8.tile([P, 512], bf, tag="m8")
                                nc.scalar.activation(out=m8[:, 0:cols],
                                                     in_=mbanks[k][:, 0:cols],
                                                     func=AF.Prelu, alpha=0.2)
                                m8s[(b, k)] = m8

                    def do_attn(b):
                        S = ps_s.tile([P, 512], fp, tag="sb", name="sb")
                        S_ts[b] = S
                        lg_ps = S[:, 0:ntile]
                        for t in range(ntile):
                            mb, mpb, mcol = m_slot(t)
                            nc.tensor.matmul(
                                out=lg_ps[:, t:t + 1],
                                lhsT=m8s[(b, mb)][mpb:mpb + c, ds(mcol, P)],
                                rhs=att2_t[mpb:mpb + c, :],
                                start=True, stop=True)
                        ex_t = wp.tile([P, ntile], fp, tag="ex")
                        nc.scalar.activation(out=ex_t[:], in_=lg_ps[:],
                                             func=AF.Exp)
                        ex_ts[b] = ex_t
                        for t in range(ntile):
                            q, i = t // tq, t % tq
                            indw = iw.tile([P, P], bf, tag="iw")
                            nc.vector.tensor_scalar(
                                out=indw[:], in0=iota_t,
                                scalar1=dC_t[:, (q * 2 + b) * tq + i:
                                             (q * 2 + b) * tq + i + 1],
                                scalar2=ex_t[:, t:t + 1],
                                op0=ALU.is_equal, op1=ALU.mult)
                            indw_ts[(b, t)] = indw

                    def do_scatter(b):
                        S = S_ts[b]
                        blk_ps = S[0:65, 256:384]
                        for t in range(ntile):
                            q, i = t // tq, t % tq
                            nc.tensor.matmul(
                                out=blk_ps,
                                lhsT=G_ts[q][:, b * tq + i, 0:c + 1],
                                rhs=indw_ts[(b, t)][:],
                                start=(t == 0), stop=(t == ntile - 1))
                        bsb = wp.tile([65, P], bf, tag="bsb")
                        nc.scalar.activation(out=bsb[:], in_=blk_ps,
                                             func=AF.Copy)
                        denc_ps = S[:, 384:385]
                        nc.tensor.matmul(out=denc_ps, lhsT=bsb[64:65, :],
                                         rhs=ones_t[64:65, 0:1],
                                         start=True, stop=True)
                        post_ps = S[:, 448:448 + c]
                        nc.tensor.matmul(out=post_ps, lhsT=bsb[0:64, :],
                                         rhs=Wl_t, start=True, stop=True)
                        dpe = wp.tile([P, 1], fp, tag="dpe")
                        nc.vector.tensor_scalar_add(out=dpe[:], in0=denc_ps,
                                                    scalar1=1e-16)
                        rec = wp.tile([P, 1], fp, tag="rec")
                        nc.vector.reciprocal(out=rec[:], in_=dpe[:])
                        sc = wp.tile([P, c], fp, tag="sc")
                        nc.vector.tensor_scalar(out=sc[:], in0=post_ps,
                                                scalar1=rec[:, 0:1],
                                                scalar2=None, op0=ALU.mult)
                        outt = wp.tile([P, c], fp, tag="outt")
                        nc.vector.tensor_tensor(out=outt[:], in0=sc[:],
                                                in1=bias_t[:], op=ALU.add)
                        nc.sync.dma_start(
                            out=dd["out"][ds((2 * pv + b) * P, P), :],
                            in_=outt[:])

                    do_trans(0)
                    while pending_sc:
                        pending_sc.pop(0)()
                    do_m(0)
                    do_attn(0)
                    do_trans(1)
                    do_scatter(0)
                    do_m(1)
                    do_attn(1)
                    pending_sc.append(lambda: do_scatter(1))

                bodies.append(pair_body)

            def run_iter(pv):
                for body in bodies:
                    body(pv)
                while pending_sc:
                    pending_sc.pop(0)()

            if unroll:
                for pv in range(npairs):
                    run_iter(pv)
            else:
                CHUNK = 12
                for s0 in range(0, npairs, CHUNK):
                    with tc.For_i(s0, min(s0 + CHUNK, npairs), 1,
                                  staggered_reset=True) as pv:
                        run_iter(pv)

    nc.compile()
    return nc, dirs


def kernel(x0, x1, edge_index, edge_attr,
           Wl_b, Wr_b, We_b, att_b, b_b,
           Wl_f, Wr_f, We_f, att_f, b_f):
    x0 = np.asarray(x0, np.float32)
    x1 = np.asarray(x1, np.float32)
    edge_attr = np.asarray(edge_attr, np.float32)
    ei = np.asarray(edge_index)
    src, dst = ei[0].astype(np.int64), ei[1].astype(np.int64)

    N, d = x0.shape
    de = edge_attr.shape[1]
    c = np.asarray(Wl_b).shape[1]
    qsize = _ceil_div(N, NQ)
    assert qsize <= 32767

    cores_b, npc, npc_pad, nblk = _prep_direction(x1, src, dst, edge_attr, NCORES)
    cores_f, _, _, _ = _prep_direction(x0, dst, src, edge_attr, NCORES)

    tq = _ceil_div(max(_max_run(cores_b, qsize, nblk),
                       _max_run(cores_f, qsize, nblk)), P)

    lay_b = _layout_direction(cores_b, nblk, tq, qsize, de)
    lay_f = _layout_direction(cores_f, nblk, tq, qsize, de)

    nc, dirs = _build_program(nblk, tq, npc_pad, qsize, de, d, c)

    def tabs(x):
        t = []
        for q in range(NQ):
            xx = x[q * qsize:(q + 1) * qsize]
            rows = np.zeros((qsize, d), np.float32)
            rows[:xx.shape[0]] = xx
            t.append(rows)
        return t

    x0t, x1t = tabs(x0), tabs(x1)
    cstb = np.zeros((P, 771), np.float32)
    cstb[:, 0:P] = np.arange(P, dtype=np.float32)[None, :]      # iota
    cstb[:, P:2 * P] = np.eye(P, dtype=np.float32)              # ident
    cstb[:, 2 * P:2 * P + 1] = 1.0                              # ones
    for di, (Wl, Wr, We, att) in enumerate(
            ((Wl_b, Wr_b, We_b, att_b), (Wl_f, Wr_f, We_f, att_f))):
        o = 257 + di * 257
        Wl = np.asarray(Wl, np.float32)
        cstb[0:d, o:o + c] = Wl
        cstb[:, o + c:o + 2 * c] = np.tile(Wl, (2, 1))
        cstb[0:d, o + 2 * c:o + 3 * c] = np.asarray(Wr, np.float32)
        cstb[0:de, o + 3 * c:o + 4 * c] = np.asarray(We, np.float32)
        cstb[:, o + 4 * c:o + 4 * c + 1] = np.tile(
            np.asarray(att, np.float32).reshape(c, 1), (2, 1))
    cstb = cstb.astype(bfloat16)
    cstf = np.zeros((P, 129), np.float32)
    cstf[:, 0:1] = np.arange(P, dtype=np.float32)[:, None]      # iotac
    cstf[:, 1:1 + c] = np.asarray(b_b, np.float32)[None, :]
    cstf[:, 1 + c:1 + 2 * c] = np.asarray(b_f, np.float32)[None, :]

    def xdT_shard(xd, k):
        lo = k * npc
        sh = xd[lo:min(lo + npc, N)]
        pad = np.zeros((npc_pad, d), np.float32)
        pad[:sh.shape[0]] = sh
        return np.ascontiguousarray(pad.T).astype(bfloat16)

    in_maps = []
    for k in range(NCORES):
        (idxw_b, doffC_b, eaT_b, doffR_b) = lay_b[k]
        (idxw_f, doffC_f, eaT_f, doffR_f) = lay_f[k]
        m = {
            "cstb": cstb, "cstf": cstf,
            "idxw_b": idxw_b, "doffC_b": doffC_b, "eaT_b": eaT_b,
            "doffR_b": doffR_b,
            "idxw_f": idxw_f, "doffC_f": doffC_f, "eaT_f": eaT_f,
            "doffR_f": doffR_f,
            "xdT_b": xdT_shard(x1, k), "xdT_f": xdT_shard(x0, k),
        }
        for q in range(NQ):
            m[f"tab{q}_b"] = x0t[q]
            m[f"tab{q}_f"] = x1t[q]
        in_maps.append(m)

    kernel.last_tq, kernel.last_nblk = tq, nblk
    kernel.last_npc_pad, kernel.last_qsize = npc_pad, qsize
    res = run_bass_kernel_spmd(nc, in_maps, list(range(NCORES)))

    out_b = np.concatenate([res.results[k]["out_b"][:npc]
                            for k in range(NCORES)])[:N]
    out_f = np.concatenate([res.results[k]["out_f"][:npc]
                            for k in range(NCORES)])[:N]
    return (out_b, out_f)
